# revision 4
# baseline (speedup 1.0000x reference)
"""Trainium2 Bass kernel for nn_D3MCELL (Multi-LSTM + cell_fn recurrence).

Axis-parallel sharding over 3 cores: core a in {0,1,2} runs axis a's stacked
LSTM and the axis-a part of cell_fn. The only cross-core traffic is one
AllReduce per timestep (t1/t2 partial sums over axes); the post-AR
softmax/combine/h_new is replicated on every core. bf16 matmuls, fp32 PSUM.

Big weights (Wx/Wh/Wg_h/Wg_p/Wilc) ship to the device as fp8e4 (scaled by
256) and are upcast to bf16 on-device in a prologue — host->device transfer
over the axon tunnel dominates wall time, not device compute.
"""
import os

os.environ.setdefault("JAX_COMPILATION_CACHE_DIR", "/root/.jax_cache")
os.environ.setdefault("JAX_PERSISTENT_CACHE_MIN_COMPILE_TIME_SECS", "0")
os.environ.setdefault("JAX_PERSISTENT_CACHE_MIN_ENTRY_SIZE_BYTES", "-1")

import numpy as np
import ml_dtypes
import jax

try:
    jax.config.update("jax_compilation_cache_dir",
                      os.environ["JAX_COMPILATION_CACHE_DIR"])
    jax.config.update("jax_persistent_cache_min_compile_time_secs", 0)
    jax.config.update("jax_persistent_cache_min_entry_size_bytes", -1)
except Exception:
    pass

import concourse.bass as bass
import concourse.mybir as mybir
import concourse.tile as tile
from concourse import bacc
from concourse.masks import make_identity
from concourse.bass_utils import run_bass_kernel_spmd

AF = mybir.ActivationFunctionType
BF16 = mybir.dt.bfloat16
FP8 = mybir.dt.float8e4
F32 = mybir.dt.float32

T, B, I, H, Z, A, L = 64, 128, 512, 512, 512, 3, 5
P = 128
NC = 3
KCH = H // P
W8SCALE = 256.0  # fp8 weights are pre-scaled by this on the host


def build(nsteps):
    nc = bacc.Bacc("TRN2", target_bir_lowering=False, debug=False,
                   num_devices=NC)
    xT_in = nc.declare_dram_parameter("xT", [T, KCH, P, P], BF16, isOutput=False)
    w1_in = nc.declare_dram_parameter("w1", [L, 2, KCH, P, 4 * H], FP8, isOutput=False)
    wg_in = nc.declare_dram_parameter("wg", [L, KCH, P, 3 * Z], FP8, isOutput=False)
    wgp_in = nc.declare_dram_parameter("wgp", [L, KCH, P, 3 * Z], FP8, isOutput=False)
    wilc_in = nc.declare_dram_parameter("wilc", [L, KCH, P, Z], FP8, isOutput=False)
    wsl_in = nc.declare_dram_parameter("wslT", [L, KCH, P, H], BF16, isOutput=False)
    wlin_in = nc.declare_dram_parameter("wlinT", [KCH, P, T], BF16, isOutput=False)
    y_out = nc.declare_dram_parameter("y", [P, T], F32, isOutput=True)

    h_hist = nc.dram_tensor("h_hist", [L, T, P, H], BF16)  # hT lhsT-ready
    c_hist = nc.dram_tensor("c_hist", [L, T, P, H], BF16)  # c batch-major
    ar_in = nc.dram_tensor("ar_in", [T, 2 * L, P, Z], BF16)
    ar_out = nc.dram_tensor("ar_out", [T, 2 * L, P, Z], BF16)
    w1d = nc.dram_tensor("w1d", [L, P, 2, KCH, 4 * H], BF16)   # upcast, pre-rearranged
    wgpd = nc.dram_tensor("wgpd", [L, P, KCH, 3 * Z], BF16)

    with tile.TileContext(nc) as tc:
        with (
            tc.tile_pool(name="const", bufs=1) as const,
            tc.tile_pool(name="state", bufs=1) as state,
            tc.tile_pool(name="wres", bufs=1) as wres,
            tc.tile_pool(name="ps1", bufs=2, space="PSUM") as ps1,
        ):
            ident = const.tile([P, P], BF16)
            make_identity(nc, ident)
            y_sb = state.tile([P, T], F32, tag="ysb")
            nc.vector.memset(y_sb, 0.0)
            hgT = state.tile([P, H], BF16, tag="hgT")
            nc.vector.memset(hgT, 0.0)
            wg = wres.tile([P, L, KCH, 3 * Z], BF16, tag="wg")
            wilc = wres.tile([P, L, KCH, Z], BF16, tag="wilc")
            wsl = wres.tile([P, L, KCH, H], BF16, tag="wsl")
            wlin = wres.tile([P, KCH, T], BF16, tag="wlin")

            # ------------- phase 0: upcast fp8 weights to bf16 -------------
            with tc.tile_pool(name="up", bufs=1) as up:
                for l in range(L):
                    t8 = up.tile([P, 2, KCH, 4 * H], FP8, tag="t8")
                    nc.sync.dma_start(t8, w1_in[l].rearrange("s k p m -> p s k m"))
                    tb = up.tile([P, 2, KCH, 4 * H], BF16, tag="tb")
                    nc.vector.tensor_scalar_mul(tb, t8, 1.0 / W8SCALE)
                    nc.sync.dma_start(w1d[l], tb)
                for l in range(L):
                    g8 = up.tile([P, KCH, 3 * Z], FP8, tag="g8")
                    nc.sync.dma_start(g8, wgp_in[l].rearrange("k p m -> p k m"))
                    gb = up.tile([P, KCH, 3 * Z], BF16, tag="gb")
                    nc.vector.tensor_scalar_mul(gb, g8, 1.0 / W8SCALE)
                    nc.sync.dma_start(wgpd[l], gb)
                    h8 = up.tile([P, KCH, 3 * Z], FP8, tag="g8")
                    nc.sync.dma_start(h8, wg_in[l].rearrange("k p m -> p k m"))
                    nc.vector.tensor_scalar_mul(wg[:, l], h8, 1.0 / W8SCALE)
                i8 = up.tile([P, L, KCH, Z], FP8, tag="i8")
                nc.sync.dma_start(i8, wilc_in[:].rearrange("l k p m -> p l k m"))
                nc.vector.tensor_scalar_mul(wilc, i8, 1.0 / W8SCALE)
                nc.sync.dma_start(wsl, wsl_in[:].rearrange("l k p m -> p l k m"))
                nc.sync.dma_start(wlin, wlin_in[:].rearrange("k p t -> p k t"))

            # ------------- phase 1: stacked LSTM, level-serial -------------
            with (
                tc.tile_pool(name="wp1", bufs=1) as wp1,
                tc.tile_pool(name="st1", bufs=1) as st1,
                tc.tile_pool(name="sb1", bufs=2) as sb1,
                tc.tile_pool(name="psA", bufs=1, space="PSUM") as psA,
            ):
                for l in range(L):
                    w1l = wp1.tile([P, 2, KCH, 4 * H], BF16, tag="w1lev")
                    nc.sync.dma_start(w1l, w1d[l])
                    hT = st1.tile([P, H], BF16, tag=f"hT_{l}")
                    nc.vector.memset(hT, 0.0)
                    cst = st1.tile([P, H], F32, tag=f"c_{l}")
                    nc.vector.memset(cst, 0.0)
                    for t in range(nsteps):
                        pg4 = [psA.tile([P, 512], F32, tag=f"pg_{g}", name=f"pg_{g}") for g in range(4)]
                        inpT = sb1.tile([P, KCH, P], BF16, tag="inpT")
                        if l == 0:
                            nc.sync.dma_start(inpT, xT_in[t].rearrange("k p m -> p k m"))
                        else:
                            nc.sync.dma_start(
                                inpT, h_hist[l - 1, t].rearrange("p (k m) -> p k m", k=KCH))
                        for k in range(KCH):
                            for g in range(4):
                                nc.tensor.matmul(pg4[g], inpT[:, k],
                                                 w1l[:, 0, k, g * 512:(g + 1) * 512],
                                                 start=(k == 0), stop=False)
                        for k in range(KCH):
                            hk = hT[:, k * P:(k + 1) * P]
                            for g in range(4):
                                nc.tensor.matmul(pg4[g], hk,
                                                 w1l[:, 1, k, g * 512:(g + 1) * 512],
                                                 start=False, stop=(k == KCH - 1))
                        si = sb1.tile([P, 512], F32, tag="si")
                        nc.scalar.activation(si, pg4[0], AF.Sigmoid)
                        sf = sb1.tile([P, 512], F32, tag="sf")
                        nc.scalar.activation(sf, pg4[1], AF.Sigmoid)
                        tg = sb1.tile([P, 512], F32, tag="tg")
                        nc.scalar.activation(tg, pg4[2], AF.Tanh)
                        so = sb1.tile([P, 512], F32, tag="so")
                        nc.scalar.activation(so, pg4[3], AF.Sigmoid)
                        fc = sb1.tile([P, 512], F32, tag="fc")
                        nc.vector.tensor_mul(fc, sf, cst)
                        ig = sb1.tile([P, 512], F32, tag="ig")
                        nc.vector.tensor_mul(ig, si, tg)
                        nc.vector.tensor_add(cst, fc, ig)
                        tch = sb1.tile([P, 512], F32, tag="tch")
                        nc.scalar.activation(tch, cst, AF.Tanh)
                        h_bf = sb1.tile([P, 512], BF16, tag="h_bf")
                        nc.vector.tensor_mul(h_bf, so, tch)
                        c_bf = sb1.tile([P, 512], BF16, tag="c_bf")
                        nc.vector.tensor_copy(c_bf, cst)
                        nc.sync.dma_start(c_hist[l, t], c_bf)
                        for k in range(KCH):
                            tp = ps1.tile([P, P], BF16, tag="tp")
                            nc.tensor.transpose(tp, h_bf[:, k * P:(k + 1) * P], ident)
                            nc.vector.tensor_copy(hT[:, k * P:(k + 1) * P], tp)
                        nc.sync.dma_start(h_hist[l, t], hT)

            # ------------- phase 3: cell_fn recurrence -------------
            with (
                tc.tile_pool(name="wgpp", bufs=2) as wgpp,
                tc.tile_pool(name="tpool", bufs=1) as tpool,
                tc.tile_pool(name="sb3", bufs=2) as sb3,
                tc.tile_pool(name="psB", bufs=1, space="PSUM") as psB,
            ):
                for t in range(nsteps):
                    tparts = tpool.tile([P, 2 * L, Z], BF16, tag="tparts")
                    for l in range(L):
                        pr3 = [psB.tile([P, 512], F32, tag=f"pr_{g}", name=f"pr_{g}") for g in range(3)]
                        shT = sb3.tile([P, H], BF16, tag="shT")
                        nc.sync.dma_start(shT, h_hist[l, t])
                        wgpl = wgpp.tile([P, KCH, 3 * Z], BF16, tag="wgpl")
                        nc.sync.dma_start(wgpl, wgpd[l])
                        for k in range(KCH):
                            for g in range(3):
                                nc.tensor.matmul(pr3[g], shT[:, k * P:(k + 1) * P],
                                                 wgpl[:, k, g * 512:(g + 1) * 512],
                                                 start=(k == 0), stop=False)
                        for k in range(KCH):
                            for g in range(3):
                                nc.tensor.matmul(pr3[g], hgT[:, k * P:(k + 1) * P],
                                                 wg[:, l, k, g * 512:(g + 1) * 512],
                                                 start=False, stop=(k == KCH - 1))
                        si3 = sb3.tile([P, 512], F32, tag="si3")
                        nc.scalar.activation(si3, pr3[0], AF.Sigmoid)
                        sf3 = sb3.tile([P, 512], F32, tag="sf3")
                        nc.scalar.activation(sf3, pr3[1], AF.Sigmoid)
                        tg3 = sb3.tile([P, 512], F32, tag="tg3")
                        nc.scalar.activation(tg3, pr3[2], AF.Tanh)
                        scc = sb3.tile([P, 512], BF16, tag="scc")
                        nc.sync.dma_start(scc, c_hist[l, t])
                        icell = sb3.tile([P, 512], BF16, tag="icell")
                        nc.vector.tensor_mul(icell, si3, scc)
                        fg = sb3.tile([P, 512], F32, tag="fg")
                        nc.vector.tensor_mul(fg, sf3, tg3)
                        ccell = sb3.tile([P, 512], BF16, tag="ccell")
                        nc.vector.tensor_add(ccell, fg, icell)
                        icT = sb3.tile([P, 512], BF16, tag="icT")
                        ccT = sb3.tile([P, 512], BF16, tag="ccT")
                        for k in range(KCH):
                            tpa = ps1.tile([P, P], BF16, tag="tp")
                            nc.tensor.transpose(tpa, icell[:, k * P:(k + 1) * P], ident)
                            nc.vector.tensor_copy(icT[:, k * P:(k + 1) * P], tpa)
                            tpb = ps1.tile([P, P], BF16, tag="tp")
                            nc.tensor.transpose(tpb, ccell[:, k * P:(k + 1) * P], ident)
                            nc.vector.tensor_copy(ccT[:, k * P:(k + 1) * P], tpb)
                        t1p = psB.tile([P, Z], F32, tag="t1p")
                        t2p = psB.tile([P, Z], F32, tag="t2p")
                        for k in range(KCH):
                            nc.tensor.matmul(t1p, icT[:, k * P:(k + 1) * P], wilc[:, l, k],
                                             start=(k == 0), stop=(k == KCH - 1))
                        for k in range(KCH):
                            nc.tensor.matmul(t2p, ccT[:, k * P:(k + 1) * P], wilc[:, l, k],
                                             start=(k == 0), stop=(k == KCH - 1))
                        nc.vector.tensor_copy(tparts[:, l], t1p)
                        nc.vector.tensor_copy(tparts[:, L + l], t2p)
                    nc.sync.dma_start(ar_in[t].rearrange("u p z -> p u z"), tparts)
                    nc.gpsimd.collective_compute(
                        "AllReduce", mybir.AluOpType.add,
                        ins=[ar_in[t]], outs=[ar_out[t]],
                        replica_groups=[list(range(NC))],
                    )
                    tsum = tpool.tile([P, 2 * L, Z], BF16, tag="tsum")
                    nc.sync.dma_start(tsum, ar_out[t].rearrange("u p z -> p u z"))
                    hn = psB.tile([P, H], F32, tag="hn")
                    for l in range(L):
                        e1 = sb3.tile([P, Z], F32, tag="e1")
                        nc.scalar.activation(e1, tsum[:, l], AF.Exp)
                        ssum = sb3.tile([P, 1], F32, tag="ssum")
                        nc.vector.reduce_sum(ssum, e1, mybir.AxisListType.X)
                        rec = sb3.tile([P, 1], F32, tag="rec")
                        nc.vector.reciprocal(rec, ssum)
                        s2 = sb3.tile([P, Z], F32, tag="s2")
                        nc.scalar.activation(s2, tsum[:, L + l], AF.Sigmoid)
                        sm = sb3.tile([P, Z], F32, tag="sm")
                        nc.vector.tensor_scalar_mul(sm, e1, rec)
                        comb = sb3.tile([P, Z], BF16, tag="comb")
                        nc.vector.tensor_mul(comb, s2, sm)
                        combT = sb3.tile([P, Z], BF16, tag="combT")
                        for k in range(KCH):
                            tpc = ps1.tile([P, P], BF16, tag="tp")
                            nc.tensor.transpose(tpc, comb[:, k * P:(k + 1) * P], ident)
                            nc.vector.tensor_copy(combT[:, k * P:(k + 1) * P], tpc)
                        for k in range(KCH):
                            nc.tensor.matmul(hn, combT[:, k * P:(k + 1) * P], wsl[:, l, k],
                                             start=(l == 0 and k == 0),
                                             stop=(l == L - 1 and k == KCH - 1))
                    hnew = sb3.tile([P, H], BF16, tag="hnew")
                    nc.vector.tensor_copy(hnew, hn)
                    for k in range(KCH):
                        tpd = ps1.tile([P, P], BF16, tag="tp")
                        nc.tensor.transpose(tpd, hnew[:, k * P:(k + 1) * P], ident)
                        nc.vector.tensor_copy(hgT[:, k * P:(k + 1) * P], tpd)
                    yp = ps1.tile([P, 1], F32, tag="tp")
                    for k in range(KCH):
                        nc.tensor.matmul(yp, hgT[:, k * P:(k + 1) * P],
                                         wlin[:, k, t:t + 1],
                                         start=(k == 0), stop=(k == KCH - 1))
                    nc.vector.tensor_copy(y_sb[:, t:t + 1], yp)
            nc.sync.dma_start(y_out[:], y_sb)
    nc.finalize()
    return nc


def _prep_inputs(x, Wx, Wh, Wg_h, Wg_p, Wilc, Wsl, Wlin):
    bf = ml_dtypes.bfloat16
    f8 = ml_dtypes.float8_e4m3
    s = W8SCALE
    xT = np.ascontiguousarray(
        x.transpose(0, 2, 1).reshape(T, KCH, P, B)).astype(bf)
    in_maps = []
    for a in range(NC):
        w1 = np.empty((L, 2, KCH, P, 4 * H), dtype=f8)
        wg = np.empty((L, KCH, P, 3 * Z), dtype=f8)
        wgp = np.empty((L, KCH, P, 3 * Z), dtype=f8)
        wilc = np.empty((L, KCH, P, Z), dtype=f8)
        for l in range(L):
            w1[l, 0] = (Wx[a, l].transpose(2, 0, 1).reshape(KCH, P, 4 * H) * s).astype(f8)
            w1[l, 1] = (Wh[a, l].transpose(2, 0, 1).reshape(KCH, P, 4 * H) * s).astype(f8)
            wg[l] = (Wg_h[l, a].transpose(2, 0, 1).reshape(KCH, P, 3 * Z) * s).astype(f8)
            wgp[l] = (Wg_p[l, a].transpose(2, 0, 1).reshape(KCH, P, 3 * Z) * s).astype(f8)
            wilc[l] = (Wilc[l, a].reshape(KCH, P, Z) * s).astype(f8)
        wslT = np.empty((L, KCH, P, H), dtype=bf)
        for l in range(L):
            wslT[l] = Wsl[:, l * Z:(l + 1) * Z].T.reshape(KCH, P, H).astype(bf)
        wlinT = Wlin[:, 0, :].T.reshape(KCH, P, T).astype(bf)
        in_maps.append(dict(xT=xT, w1=w1, wg=wg, wgp=wgp, wilc=wilc,
                            wslT=wslT, wlinT=wlinT))
    return in_maps


def kernel(x, Wx, Wh, b_lstm, Wg_h, Wg_p, bg, Wilc, bilc, Wsl, bsl, Wlin, blin,
           _nsteps=T):
    x = np.asarray(x, np.float32)
    for nm, b in (("b_lstm", b_lstm), ("bg", bg), ("bilc", bilc), ("bsl", bsl),
                  ("blin", blin)):
        assert not np.any(np.asarray(b)), f"nonzero bias {nm} unsupported"
    in_maps = _prep_inputs(x, np.asarray(Wx, np.float32), np.asarray(Wh, np.float32),
                           np.asarray(Wg_h, np.float32), np.asarray(Wg_p, np.float32),
                           np.asarray(Wilc, np.float32), np.asarray(Wsl, np.float32),
                           np.asarray(Wlin, np.float32))
    nc = build(_nsteps)
    res = run_bass_kernel_spmd(nc, in_maps, list(range(NC)))
    y = np.asarray(res.results[0]["y"], np.float32)  # (B, T)
    return np.ascontiguousarray(y.T[:, :, None])  # (T, B, 1)


# revision 5
# speedup vs baseline: 7.7666x; 7.7666x over previous
"""Trainium2 Bass kernel for nn_D3MCELL (Multi-LSTM + cell_fn recurrence).

Axis-parallel sharding over 3 cores: core a in {0,1,2} runs axis a's stacked
LSTM and the axis-a part of cell_fn. The only cross-core traffic is one
AllReduce per timestep (t1/t2 partial sums over axes); the post-AR
softmax/combine/h_new is replicated on every core. bf16 matmuls, fp32 PSUM.

Big weights (Wx/Wh/Wg_h/Wg_p/Wilc) ship to the device as fp8e4 (scaled by
256) and are upcast to bf16 on-device in a prologue — host->device transfer
over the axon tunnel dominates wall time, not device compute.
"""
import os

os.environ.setdefault("JAX_COMPILATION_CACHE_DIR", "/root/.jax_cache")
os.environ.setdefault("JAX_PERSISTENT_CACHE_MIN_COMPILE_TIME_SECS", "0")
os.environ.setdefault("JAX_PERSISTENT_CACHE_MIN_ENTRY_SIZE_BYTES", "-1")
os.environ["BASS_DISABLE_FRAME_TO_TRACEBACK"] = "1"

import numpy as np
import ml_dtypes
import jax

try:
    jax.config.update("jax_compilation_cache_dir",
                      os.environ["JAX_COMPILATION_CACHE_DIR"])
    jax.config.update("jax_persistent_cache_min_compile_time_secs", 0)
    jax.config.update("jax_persistent_cache_min_entry_size_bytes", -1)
except Exception:
    pass

import concourse.bass as bass
import concourse.mybir as mybir
import concourse.tile as tile
from concourse import bacc
from concourse.masks import make_identity
from concourse.bass_utils import run_bass_kernel_spmd

AF = mybir.ActivationFunctionType
BF16 = mybir.dt.bfloat16
FP8 = mybir.dt.float8e4
F32 = mybir.dt.float32

T, B, I, H, Z, A, L = 64, 128, 512, 512, 512, 3, 5
P = 128
NC = 3
KCH = H // P
W8SCALE = 256.0  # fp8 weights are pre-scaled by this on the host


def build(nsteps):
    nc = bacc.Bacc("TRN2", target_bir_lowering=False, debug=False,
                   num_devices=NC)
    xT_in = nc.declare_dram_parameter("xT", [T, KCH, P, P], BF16, isOutput=False)
    w1_in = nc.declare_dram_parameter("w1", [L, 2, KCH, P, 4 * H], FP8, isOutput=False)
    wg_in = nc.declare_dram_parameter("wg", [L, KCH, P, 3 * Z], FP8, isOutput=False)
    wgp_in = nc.declare_dram_parameter("wgp", [L, KCH, P, 3 * Z], FP8, isOutput=False)
    wilc_in = nc.declare_dram_parameter("wilc", [L, KCH, P, Z], FP8, isOutput=False)
    wsl_in = nc.declare_dram_parameter("wslT", [L, KCH, P, H], BF16, isOutput=False)
    wlin_in = nc.declare_dram_parameter("wlinT", [KCH, P, T], BF16, isOutput=False)
    y_out = nc.declare_dram_parameter("y", [P, T], F32, isOutput=True)

    h_hist = nc.dram_tensor("h_hist", [L, T, P, H], BF16)  # hT lhsT-ready
    c_hist = nc.dram_tensor("c_hist", [L, T, P, H], BF16)  # c batch-major
    ar_in = nc.dram_tensor("ar_in", [T, 2 * L, P, Z], BF16)
    ar_out = nc.dram_tensor("ar_out", [T, 2 * L, P, Z], BF16)
    w1d = nc.dram_tensor("w1d", [L, P, 2, KCH, 4 * H], BF16)   # upcast, pre-rearranged
    wgpd = nc.dram_tensor("wgpd", [L, P, KCH, 3 * Z], BF16)

    with tile.TileContext(nc) as tc:
        with (
            tc.tile_pool(name="const", bufs=1) as const,
            tc.tile_pool(name="state", bufs=1) as state,
            tc.tile_pool(name="wres", bufs=1) as wres,
            tc.tile_pool(name="ps1", bufs=2, space="PSUM") as ps1,
        ):
            ident = const.tile([P, P], BF16)
            make_identity(nc, ident)
            y_sb = state.tile([P, T], F32, tag="ysb")
            nc.vector.memset(y_sb, 0.0)
            hgT = state.tile([P, H], BF16, tag="hgT")
            nc.vector.memset(hgT, 0.0)
            wg = wres.tile([P, L, KCH, 3 * Z], BF16, tag="wg")
            wilc = wres.tile([P, L, KCH, Z], BF16, tag="wilc")
            wsl = wres.tile([P, L, KCH, H], BF16, tag="wsl")
            wlin = wres.tile([P, KCH, T], BF16, tag="wlin")

            # ------------- phase 0: upcast fp8 weights to bf16 -------------
            with tc.tile_pool(name="up", bufs=1) as up:
                for l in range(L):
                    t8 = up.tile([P, 2, KCH, 4 * H], FP8, tag="t8")
                    nc.sync.dma_start(t8, w1_in[l].rearrange("s k p m -> p s k m"))
                    tb = up.tile([P, 2, KCH, 4 * H], BF16, tag="tb")
                    nc.vector.tensor_scalar_mul(tb, t8, 1.0 / W8SCALE)
                    nc.sync.dma_start(w1d[l], tb)
                for l in range(L):
                    g8 = up.tile([P, KCH, 3 * Z], FP8, tag="g8")
                    nc.sync.dma_start(g8, wgp_in[l].rearrange("k p m -> p k m"))
                    gb = up.tile([P, KCH, 3 * Z], BF16, tag="gb")
                    nc.vector.tensor_scalar_mul(gb, g8, 1.0 / W8SCALE)
                    nc.sync.dma_start(wgpd[l], gb)
                    h8 = up.tile([P, KCH, 3 * Z], FP8, tag="g8")
                    nc.sync.dma_start(h8, wg_in[l].rearrange("k p m -> p k m"))
                    nc.vector.tensor_scalar_mul(wg[:, l], h8, 1.0 / W8SCALE)
                i8 = up.tile([P, L, KCH, Z], FP8, tag="i8")
                nc.sync.dma_start(i8, wilc_in[:].rearrange("l k p m -> p l k m"))
                nc.vector.tensor_scalar_mul(wilc, i8, 1.0 / W8SCALE)
                nc.sync.dma_start(wsl, wsl_in[:].rearrange("l k p m -> p l k m"))
                nc.sync.dma_start(wlin, wlin_in[:].rearrange("k p t -> p k t"))

            # ------------- phase 1: stacked LSTM, level-serial -------------
            with (
                tc.tile_pool(name="wp1", bufs=1) as wp1,
                tc.tile_pool(name="st1", bufs=1) as st1,
                tc.tile_pool(name="sb1", bufs=2) as sb1,
                tc.tile_pool(name="psA", bufs=1, space="PSUM") as psA,
            ):
                for l in range(L):
                    w1l = wp1.tile([P, 2, KCH, 4 * H], BF16, tag="w1lev")
                    nc.sync.dma_start(w1l, w1d[l])
                    hT = st1.tile([P, H], BF16, tag=f"hT_{l}")
                    nc.vector.memset(hT, 0.0)
                    cst = st1.tile([P, H], F32, tag=f"c_{l}")
                    nc.vector.memset(cst, 0.0)
                    for t in range(nsteps):
                        pg4 = [psA.tile([P, 512], F32, tag=f"pg_{g}", name=f"pg_{g}") for g in range(4)]
                        inpT = sb1.tile([P, KCH, P], BF16, tag="inpT")
                        if l == 0:
                            nc.sync.dma_start(inpT, xT_in[t].rearrange("k p m -> p k m"))
                        else:
                            nc.sync.dma_start(
                                inpT, h_hist[l - 1, t].rearrange("p (k m) -> p k m", k=KCH))
                        for k in range(KCH):
                            for g in range(4):
                                nc.tensor.matmul(pg4[g], inpT[:, k],
                                                 w1l[:, 0, k, g * 512:(g + 1) * 512],
                                                 start=(k == 0), stop=False)
                        for k in range(KCH):
                            hk = hT[:, k * P:(k + 1) * P]
                            for g in range(4):
                                nc.tensor.matmul(pg4[g], hk,
                                                 w1l[:, 1, k, g * 512:(g + 1) * 512],
                                                 start=False, stop=(k == KCH - 1))
                        si = sb1.tile([P, 512], F32, tag="si")
                        nc.scalar.activation(si, pg4[0], AF.Sigmoid)
                        sf = sb1.tile([P, 512], F32, tag="sf")
                        nc.scalar.activation(sf, pg4[1], AF.Sigmoid)
                        tg = sb1.tile([P, 512], F32, tag="tg")
                        nc.scalar.activation(tg, pg4[2], AF.Tanh)
                        so = sb1.tile([P, 512], F32, tag="so")
                        nc.scalar.activation(so, pg4[3], AF.Sigmoid)
                        fc = sb1.tile([P, 512], F32, tag="fc")
                        nc.vector.tensor_mul(fc, sf, cst)
                        ig = sb1.tile([P, 512], F32, tag="ig")
                        nc.vector.tensor_mul(ig, si, tg)
                        nc.vector.tensor_add(cst, fc, ig)
                        tch = sb1.tile([P, 512], F32, tag="tch")
                        nc.scalar.activation(tch, cst, AF.Tanh)
                        h_bf = sb1.tile([P, 512], BF16, tag="h_bf")
                        nc.vector.tensor_mul(h_bf, so, tch)
                        c_bf = sb1.tile([P, 512], BF16, tag="c_bf")
                        nc.vector.tensor_copy(c_bf, cst)
                        nc.sync.dma_start(c_hist[l, t], c_bf)
                        for k in range(KCH):
                            tp = ps1.tile([P, P], BF16, tag="tp")
                            nc.tensor.transpose(tp, h_bf[:, k * P:(k + 1) * P], ident)
                            nc.vector.tensor_copy(hT[:, k * P:(k + 1) * P], tp)
                        nc.sync.dma_start(h_hist[l, t], hT)

            # ------------- phase 3: cell_fn recurrence -------------
            with (
                tc.tile_pool(name="wgpp", bufs=2) as wgpp,
                tc.tile_pool(name="tpool", bufs=1) as tpool,
                tc.tile_pool(name="sb3", bufs=2) as sb3,
                tc.tile_pool(name="psB", bufs=1, space="PSUM") as psB,
            ):
                for t in range(nsteps):
                    tparts = tpool.tile([P, 2 * L, Z], BF16, tag="tparts")
                    for l in range(L):
                        pr3 = [psB.tile([P, 512], F32, tag=f"pr_{g}", name=f"pr_{g}") for g in range(3)]
                        shT = sb3.tile([P, H], BF16, tag="shT")
                        nc.sync.dma_start(shT, h_hist[l, t])
                        wgpl = wgpp.tile([P, KCH, 3 * Z], BF16, tag="wgpl")
                        nc.sync.dma_start(wgpl, wgpd[l])
                        for k in range(KCH):
                            for g in range(3):
                                nc.tensor.matmul(pr3[g], shT[:, k * P:(k + 1) * P],
                                                 wgpl[:, k, g * 512:(g + 1) * 512],
                                                 start=(k == 0), stop=False)
                        for k in range(KCH):
                            for g in range(3):
                                nc.tensor.matmul(pr3[g], hgT[:, k * P:(k + 1) * P],
                                                 wg[:, l, k, g * 512:(g + 1) * 512],
                                                 start=False, stop=(k == KCH - 1))
                        si3 = sb3.tile([P, 512], F32, tag="si3")
                        nc.scalar.activation(si3, pr3[0], AF.Sigmoid)
                        sf3 = sb3.tile([P, 512], F32, tag="sf3")
                        nc.scalar.activation(sf3, pr3[1], AF.Sigmoid)
                        tg3 = sb3.tile([P, 512], F32, tag="tg3")
                        nc.scalar.activation(tg3, pr3[2], AF.Tanh)
                        scc = sb3.tile([P, 512], BF16, tag="scc")
                        nc.sync.dma_start(scc, c_hist[l, t])
                        icell = sb3.tile([P, 512], BF16, tag="icell")
                        nc.vector.tensor_mul(icell, si3, scc)
                        fg = sb3.tile([P, 512], F32, tag="fg")
                        nc.vector.tensor_mul(fg, sf3, tg3)
                        ccell = sb3.tile([P, 512], BF16, tag="ccell")
                        nc.vector.tensor_add(ccell, fg, icell)
                        icT = sb3.tile([P, 512], BF16, tag="icT")
                        ccT = sb3.tile([P, 512], BF16, tag="ccT")
                        for k in range(KCH):
                            tpa = ps1.tile([P, P], BF16, tag="tp")
                            nc.tensor.transpose(tpa, icell[:, k * P:(k + 1) * P], ident)
                            nc.vector.tensor_copy(icT[:, k * P:(k + 1) * P], tpa)
                            tpb = ps1.tile([P, P], BF16, tag="tp")
                            nc.tensor.transpose(tpb, ccell[:, k * P:(k + 1) * P], ident)
                            nc.vector.tensor_copy(ccT[:, k * P:(k + 1) * P], tpb)
                        t1p = psB.tile([P, Z], F32, tag="t1p")
                        t2p = psB.tile([P, Z], F32, tag="t2p")
                        for k in range(KCH):
                            nc.tensor.matmul(t1p, icT[:, k * P:(k + 1) * P], wilc[:, l, k],
                                             start=(k == 0), stop=(k == KCH - 1))
                        for k in range(KCH):
                            nc.tensor.matmul(t2p, ccT[:, k * P:(k + 1) * P], wilc[:, l, k],
                                             start=(k == 0), stop=(k == KCH - 1))
                        nc.vector.tensor_copy(tparts[:, l], t1p)
                        nc.vector.tensor_copy(tparts[:, L + l], t2p)
                    nc.sync.dma_start(ar_in[t].rearrange("u p z -> p u z"), tparts)
                    nc.gpsimd.collective_compute(
                        "AllReduce", mybir.AluOpType.add,
                        ins=[ar_in[t]], outs=[ar_out[t]],
                        replica_groups=[list(range(NC))],
                    )
                    tsum = tpool.tile([P, 2 * L, Z], BF16, tag="tsum")
                    nc.sync.dma_start(tsum, ar_out[t].rearrange("u p z -> p u z"))
                    hn = psB.tile([P, H], F32, tag="hn")
                    for l in range(L):
                        e1 = sb3.tile([P, Z], F32, tag="e1")
                        nc.scalar.activation(e1, tsum[:, l], AF.Exp)
                        ssum = sb3.tile([P, 1], F32, tag="ssum")
                        nc.vector.reduce_sum(ssum, e1, mybir.AxisListType.X)
                        rec = sb3.tile([P, 1], F32, tag="rec")
                        nc.vector.reciprocal(rec, ssum)
                        s2 = sb3.tile([P, Z], F32, tag="s2")
                        nc.scalar.activation(s2, tsum[:, L + l], AF.Sigmoid)
                        sm = sb3.tile([P, Z], F32, tag="sm")
                        nc.vector.tensor_scalar_mul(sm, e1, rec)
                        comb = sb3.tile([P, Z], BF16, tag="comb")
                        nc.vector.tensor_mul(comb, s2, sm)
                        combT = sb3.tile([P, Z], BF16, tag="combT")
                        for k in range(KCH):
                            tpc = ps1.tile([P, P], BF16, tag="tp")
                            nc.tensor.transpose(tpc, comb[:, k * P:(k + 1) * P], ident)
                            nc.vector.tensor_copy(combT[:, k * P:(k + 1) * P], tpc)
                        for k in range(KCH):
                            nc.tensor.matmul(hn, combT[:, k * P:(k + 1) * P], wsl[:, l, k],
                                             start=(l == 0 and k == 0),
                                             stop=(l == L - 1 and k == KCH - 1))
                    hnew = sb3.tile([P, H], BF16, tag="hnew")
                    nc.vector.tensor_copy(hnew, hn)
                    for k in range(KCH):
                        tpd = ps1.tile([P, P], BF16, tag="tp")
                        nc.tensor.transpose(tpd, hnew[:, k * P:(k + 1) * P], ident)
                        nc.vector.tensor_copy(hgT[:, k * P:(k + 1) * P], tpd)
                    yp = ps1.tile([P, 1], F32, tag="tp")
                    for k in range(KCH):
                        nc.tensor.matmul(yp, hgT[:, k * P:(k + 1) * P],
                                         wlin[:, k, t:t + 1],
                                         start=(k == 0), stop=(k == KCH - 1))
                    nc.vector.tensor_copy(y_sb[:, t:t + 1], yp)
            nc.sync.dma_start(y_out[:], y_sb)
    nc.finalize()
    return nc


def _prep_inputs(x, Wx, Wh, Wg_h, Wg_p, Wilc, Wsl, Wlin):
    bf = ml_dtypes.bfloat16
    f8 = ml_dtypes.float8_e4m3
    s = W8SCALE
    xT = np.ascontiguousarray(
        x.transpose(0, 2, 1).reshape(T, KCH, P, B)).astype(bf)
    in_maps = []
    for a in range(NC):
        w1 = np.empty((L, 2, KCH, P, 4 * H), dtype=f8)
        wg = np.empty((L, KCH, P, 3 * Z), dtype=f8)
        wgp = np.empty((L, KCH, P, 3 * Z), dtype=f8)
        wilc = np.empty((L, KCH, P, Z), dtype=f8)
        for l in range(L):
            w1[l, 0] = (Wx[a, l].transpose(2, 0, 1).reshape(KCH, P, 4 * H) * s).astype(f8)
            w1[l, 1] = (Wh[a, l].transpose(2, 0, 1).reshape(KCH, P, 4 * H) * s).astype(f8)
            wg[l] = (Wg_h[l, a].transpose(2, 0, 1).reshape(KCH, P, 3 * Z) * s).astype(f8)
            wgp[l] = (Wg_p[l, a].transpose(2, 0, 1).reshape(KCH, P, 3 * Z) * s).astype(f8)
            wilc[l] = (Wilc[l, a].reshape(KCH, P, Z) * s).astype(f8)
        wslT = np.empty((L, KCH, P, H), dtype=bf)
        for l in range(L):
            wslT[l] = Wsl[:, l * Z:(l + 1) * Z].T.reshape(KCH, P, H).astype(bf)
        wlinT = Wlin[:, 0, :].T.reshape(KCH, P, T).astype(bf)
        in_maps.append(dict(xT=xT, w1=w1, wg=wg, wgp=wgp, wilc=wilc,
                            wslT=wslT, wlinT=wlinT))
    return in_maps


def kernel(x, Wx, Wh, b_lstm, Wg_h, Wg_p, bg, Wilc, bilc, Wsl, bsl, Wlin, blin,
           _nsteps=T):
    x = np.asarray(x, np.float32)
    for nm, b in (("b_lstm", b_lstm), ("bg", bg), ("bilc", bilc), ("bsl", bsl),
                  ("blin", blin)):
        assert not np.any(np.asarray(b)), f"nonzero bias {nm} unsupported"
    in_maps = _prep_inputs(x, np.asarray(Wx, np.float32), np.asarray(Wh, np.float32),
                           np.asarray(Wg_h, np.float32), np.asarray(Wg_p, np.float32),
                           np.asarray(Wilc, np.float32), np.asarray(Wsl, np.float32),
                           np.asarray(Wlin, np.float32))
    nc = build(_nsteps)
    res = run_bass_kernel_spmd(nc, in_maps, list(range(NC)))
    y = np.asarray(res.results[0]["y"], np.float32)  # (B, T)
    return np.ascontiguousarray(y.T[:, :, None])  # (T, B, 1)


# revision 14
# speedup vs baseline: 8.9777x; 1.1559x over previous
"""Trainium2 Bass kernel for nn_D3MCELL (Multi-LSTM + cell_fn recurrence).

Axis-parallel sharding over 3 cores: core a in {0,1,2} runs axis a's stacked
LSTM and the axis-a part of cell_fn. The only cross-core traffic is one
AllReduce per timestep (t1/t2 partial sums over axes); the post-AR
softmax/combine/h_new is replicated on every core. bf16 matmuls, fp32 PSUM.

Big weights (Wx/Wh/Wg_h/Wg_p/Wilc) ship to the device as fp8e4 (scaled by
256) and are upcast to bf16 on-device in a prologue — host->device transfer
over the axon tunnel dominates wall time, not device compute.
"""
import os

os.environ.setdefault("JAX_COMPILATION_CACHE_DIR", "/root/.jax_cache")
os.environ.setdefault("JAX_PERSISTENT_CACHE_MIN_COMPILE_TIME_SECS", "0")
os.environ.setdefault("JAX_PERSISTENT_CACHE_MIN_ENTRY_SIZE_BYTES", "-1")
os.environ["BASS_DISABLE_FRAME_TO_TRACEBACK"] = "1"

import numpy as np
import ml_dtypes
import jax

try:
    jax.config.update("jax_compilation_cache_dir",
                      os.environ["JAX_COMPILATION_CACHE_DIR"])
    jax.config.update("jax_persistent_cache_min_compile_time_secs", 0)
    jax.config.update("jax_persistent_cache_min_entry_size_bytes", -1)
except Exception:
    pass

import concourse.bass as bass
import concourse.mybir as mybir
import concourse.tile as tile
from concourse import bacc
from concourse.masks import make_identity
from concourse.bass_utils import run_bass_kernel_spmd

AF = mybir.ActivationFunctionType
BF16 = mybir.dt.bfloat16
FP8 = mybir.dt.float8e4
F32 = mybir.dt.float32

T, B, I, H, Z, A, L = 64, 128, 512, 512, 512, 3, 5
P = 128
NC = 3
KCH = H // P
W8SCALE = 256.0  # fp8 weights are pre-scaled by this on the host
TSH = (T + NC - 1) // NC   # 22: x timesteps per core (sharded, AllGathered)
LSH = (L + NC - 1) // NC   # 2: Wsl levels per core


def build(nsteps):
    nc = bacc.Bacc("TRN2", target_bir_lowering=False, debug=False,
                   num_devices=NC)
    xs_in = nc.declare_dram_parameter("xs", [TSH, KCH, P, P], BF16, isOutput=False)
    w1_in = nc.declare_dram_parameter("w1", [L, 2, KCH, P, 4 * H], FP8, isOutput=False)
    wg_in = nc.declare_dram_parameter("wg", [L, KCH, P, 3 * Z], FP8, isOutput=False)
    wgp_in = nc.declare_dram_parameter("wgp", [L, KCH, P, 3 * Z], FP8, isOutput=False)
    wilc_in = nc.declare_dram_parameter("wilc", [L, KCH, P, Z], FP8, isOutput=False)
    wsls_in = nc.declare_dram_parameter("wsls", [LSH, KCH, P, H], BF16, isOutput=False)
    wlin_in = nc.declare_dram_parameter("wlinT", [KCH, P, T], BF16, isOutput=False)
    y_out = nc.declare_dram_parameter("y", [P, T], F32, isOutput=True)
    xs_st = nc.dram_tensor("xs_st", [TSH, KCH, P, P], BF16)
    wsls_st = nc.dram_tensor("wsls_st", [LSH, KCH, P, H], BF16)
    xg = nc.dram_tensor("xg", [NC * TSH, KCH, P, P], BF16)
    wslg = nc.dram_tensor("wslg", [NC * LSH, KCH, P, H], BF16)

    h_hist = nc.dram_tensor("h_hist", [L, T, P, H], BF16)  # hT lhsT-ready
    c_hist = nc.dram_tensor("c_hist", [L, T, P, H], BF16)  # c batch-major
    ar_in = nc.dram_tensor("ar_in", [T, 2 * L, P, Z], BF16)
    ar_out = nc.dram_tensor("ar_out", [T, 2 * L, P, Z], BF16)
    w1d = nc.dram_tensor("w1d", [L, P, 2, KCH, 4 * H], BF16)   # upcast, pre-rearranged
    wgpd = nc.dram_tensor("wgpd", [L, P, KCH, 3 * Z], BF16)

    with tile.TileContext(nc) as tc:
        with (
            tc.tile_pool(name="const", bufs=1) as const,
            tc.tile_pool(name="state", bufs=1) as state,
            tc.tile_pool(name="wres", bufs=1) as wres,
            tc.tile_pool(name="ps1", bufs=2, space="PSUM") as ps1,
        ):
            ident = const.tile([P, P], BF16)
            make_identity(nc, ident)
            y_sb = state.tile([P, T], F32, tag="ysb")
            nc.vector.memset(y_sb, 0.0)
            hgT = state.tile([P, H], BF16, tag="hgT")
            nc.vector.memset(hgT, 0.0)
            wg = wres.tile([P, L, KCH, 3 * Z], BF16, tag="wg")
            wilc = wres.tile([P, L, KCH, Z], BF16, tag="wilc")
            wsl = wres.tile([P, L, KCH, H], BF16, tag="wsl")
            wlin = wres.tile([P, KCH, T], BF16, tag="wlin")

            # ------------- phase 0a: AllGather sharded x / Wsl -------------
            nc.sync.dma_start(xs_st[:], xs_in[:])
            nc.sync.dma_start(wsls_st[:], wsls_in[:])
            nc.gpsimd.collective_compute(
                "AllGather", mybir.AluOpType.bypass,
                ins=[xs_st[:]], outs=[xg[:]],
                replica_groups=[list(range(NC))],
            )
            nc.gpsimd.collective_compute(
                "AllGather", mybir.AluOpType.bypass,
                ins=[wsls_st[:]], outs=[wslg[:]],
                replica_groups=[list(range(NC))],
            )

            # ------------- phase 0: upcast fp8 weights to bf16 -------------
            with tc.tile_pool(name="up", bufs=1) as up:
                for l in range(L):
                    t8 = up.tile([P, 2, KCH, 4 * H], FP8, tag="t8")
                    nc.sync.dma_start(t8, w1_in[l].rearrange("s k p m -> p s k m"))
                    tb = up.tile([P, 2, KCH, 4 * H], BF16, tag="tb")
                    nc.vector.tensor_scalar_mul(tb, t8, 1.0 / W8SCALE)
                    nc.sync.dma_start(w1d[l], tb)
                for l in range(L):
                    g8 = up.tile([P, KCH, 3 * Z], FP8, tag="g8")
                    nc.sync.dma_start(g8, wgp_in[l].rearrange("k p m -> p k m"))
                    gb = up.tile([P, KCH, 3 * Z], BF16, tag="gb")
                    nc.vector.tensor_scalar_mul(gb, g8, 1.0 / W8SCALE)
                    nc.sync.dma_start(wgpd[l], gb)
                    h8 = up.tile([P, KCH, 3 * Z], FP8, tag="g8")
                    nc.sync.dma_start(h8, wg_in[l].rearrange("k p m -> p k m"))
                    nc.vector.tensor_scalar_mul(wg[:, l], h8, 1.0 / W8SCALE)
                i8 = up.tile([P, L, KCH, Z], FP8, tag="i8")
                nc.sync.dma_start(i8, wilc_in[:].rearrange("l k p m -> p l k m"))
                nc.vector.tensor_scalar_mul(wilc, i8, 1.0 / W8SCALE)
                for l in range(L):
                    nc.sync.dma_start(wsl[:, l], wslg[l].rearrange("k p m -> p k m"))
                nc.sync.dma_start(wlin, wlin_in[:].rearrange("k p t -> p k t"))

            # ------------- phase 1: stacked LSTM, level-serial -------------
            with (
                tc.tile_pool(name="wp1", bufs=1) as wp1,
                tc.tile_pool(name="st1", bufs=1) as st1,
                tc.tile_pool(name="sb1", bufs=2) as sb1,
                tc.tile_pool(name="psA", bufs=1, space="PSUM") as psA,
            ):
                for l in range(L):
                    w1l = wp1.tile([P, 2, KCH, 4 * H], BF16, tag="w1lev")
                    nc.sync.dma_start(w1l, w1d[l])
                    hT = st1.tile([P, H], BF16, tag=f"hT_{l}")
                    nc.vector.memset(hT, 0.0)
                    cst = st1.tile([P, H], F32, tag=f"c_{l}")
                    nc.vector.memset(cst, 0.0)
                    for t in range(nsteps):
                        pg4 = [psA.tile([P, 512], F32, tag=f"pg_{g}", name=f"pg_{g}") for g in range(4)]
                        inpT = sb1.tile([P, KCH, P], BF16, tag="inpT")
                        if l == 0:
                            nc.sync.dma_start(inpT, xg[t].rearrange("k p m -> p k m"))
                        else:
                            nc.sync.dma_start(
                                inpT, h_hist[l - 1, t].rearrange("p (k m) -> p k m", k=KCH))
                        for k in range(KCH):
                            for g in range(4):
                                nc.tensor.matmul(pg4[g], inpT[:, k],
                                                 w1l[:, 0, k, g * 512:(g + 1) * 512],
                                                 start=(k == 0), stop=False)
                        for k in range(KCH):
                            hk = hT[:, k * P:(k + 1) * P]
                            for g in range(4):
                                nc.tensor.matmul(pg4[g], hk,
                                                 w1l[:, 1, k, g * 512:(g + 1) * 512],
                                                 start=False, stop=(k == KCH - 1))
                        si = sb1.tile([P, 512], F32, tag="si")
                        nc.scalar.activation(si, pg4[0], AF.Sigmoid)
                        sf = sb1.tile([P, 512], F32, tag="sf")
                        nc.scalar.activation(sf, pg4[1], AF.Sigmoid)
                        tg = sb1.tile([P, 512], F32, tag="tg")
                        nc.scalar.activation(tg, pg4[2], AF.Tanh)
                        so = sb1.tile([P, 512], F32, tag="so")
                        nc.scalar.activation(so, pg4[3], AF.Sigmoid)
                        fc = sb1.tile([P, 512], F32, tag="fc")
                        nc.vector.tensor_mul(fc, sf, cst)
                        ig = sb1.tile([P, 512], F32, tag="ig")
                        nc.vector.tensor_mul(ig, si, tg)
                        nc.vector.tensor_add(cst, fc, ig)
                        tch = sb1.tile([P, 512], F32, tag="tch")
                        nc.scalar.activation(tch, cst, AF.Tanh)
                        h_bf = sb1.tile([P, 512], BF16, tag="h_bf")
                        nc.vector.tensor_mul(h_bf, so, tch)
                        c_bf = sb1.tile([P, 512], BF16, tag="c_bf")
                        nc.vector.tensor_copy(c_bf, cst)
                        nc.sync.dma_start(c_hist[l, t], c_bf)
                        for k in range(KCH):
                            tp = ps1.tile([P, P], BF16, tag="tp")
                            nc.tensor.transpose(tp, h_bf[:, k * P:(k + 1) * P], ident)
                            nc.vector.tensor_copy(hT[:, k * P:(k + 1) * P], tp)
                        nc.sync.dma_start(h_hist[l, t], hT)

            # ------------- phase 3: cell_fn recurrence -------------
            with (
                tc.tile_pool(name="wgpp", bufs=2) as wgpp,
                tc.tile_pool(name="tpool", bufs=1) as tpool,
                tc.tile_pool(name="sb3", bufs=2) as sb3,
                tc.tile_pool(name="psB", bufs=1, space="PSUM") as psB,
            ):
                for t in range(nsteps):
                    tparts = tpool.tile([P, 2 * L, Z], BF16, tag="tparts")
                    for l in range(L):
                        pr3 = [psB.tile([P, 512], F32, tag=f"pr_{g}", name=f"pr_{g}") for g in range(3)]
                        shT = sb3.tile([P, H], BF16, tag="shT")
                        nc.sync.dma_start(shT, h_hist[l, t])
                        wgpl = wgpp.tile([P, KCH, 3 * Z], BF16, tag="wgpl")
                        nc.sync.dma_start(wgpl, wgpd[l])
                        for k in range(KCH):
                            for g in range(3):
                                nc.tensor.matmul(pr3[g], shT[:, k * P:(k + 1) * P],
                                                 wgpl[:, k, g * 512:(g + 1) * 512],
                                                 start=(k == 0), stop=False)
                        for k in range(KCH):
                            for g in range(3):
                                nc.tensor.matmul(pr3[g], hgT[:, k * P:(k + 1) * P],
                                                 wg[:, l, k, g * 512:(g + 1) * 512],
                                                 start=False, stop=(k == KCH - 1))
                        si3 = sb3.tile([P, 512], F32, tag="si3")
                        nc.scalar.activation(si3, pr3[0], AF.Sigmoid)
                        sf3 = sb3.tile([P, 512], F32, tag="sf3")
                        nc.scalar.activation(sf3, pr3[1], AF.Sigmoid)
                        tg3 = sb3.tile([P, 512], F32, tag="tg3")
                        nc.scalar.activation(tg3, pr3[2], AF.Tanh)
                        scc = sb3.tile([P, 512], BF16, tag="scc")
                        nc.sync.dma_start(scc, c_hist[l, t])
                        icell = sb3.tile([P, 512], BF16, tag="icell")
                        nc.vector.tensor_mul(icell, si3, scc)
                        fg = sb3.tile([P, 512], F32, tag="fg")
                        nc.vector.tensor_mul(fg, sf3, tg3)
                        ccell = sb3.tile([P, 512], BF16, tag="ccell")
                        nc.vector.tensor_add(ccell, fg, icell)
                        icT = sb3.tile([P, 512], BF16, tag="icT")
                        ccT = sb3.tile([P, 512], BF16, tag="ccT")
                        for k in range(KCH):
                            tpa = ps1.tile([P, P], BF16, tag="tp")
                            nc.tensor.transpose(tpa, icell[:, k * P:(k + 1) * P], ident)
                            nc.vector.tensor_copy(icT[:, k * P:(k + 1) * P], tpa)
                            tpb = ps1.tile([P, P], BF16, tag="tp")
                            nc.tensor.transpose(tpb, ccell[:, k * P:(k + 1) * P], ident)
                            nc.vector.tensor_copy(ccT[:, k * P:(k + 1) * P], tpb)
                        t1p = psB.tile([P, Z], F32, tag="t1p")
                        t2p = psB.tile([P, Z], F32, tag="t2p")
                        for k in range(KCH):
                            nc.tensor.matmul(t1p, icT[:, k * P:(k + 1) * P], wilc[:, l, k],
                                             start=(k == 0), stop=(k == KCH - 1))
                        for k in range(KCH):
                            nc.tensor.matmul(t2p, ccT[:, k * P:(k + 1) * P], wilc[:, l, k],
                                             start=(k == 0), stop=(k == KCH - 1))
                        nc.vector.tensor_copy(tparts[:, l], t1p)
                        nc.vector.tensor_copy(tparts[:, L + l], t2p)
                    nc.sync.dma_start(ar_in[t].rearrange("u p z -> p u z"), tparts)
                    nc.gpsimd.collective_compute(
                        "AllReduce", mybir.AluOpType.add,
                        ins=[ar_in[t]], outs=[ar_out[t]],
                        replica_groups=[list(range(NC))],
                    )
                    tsum = tpool.tile([P, 2 * L, Z], BF16, tag="tsum")
                    nc.sync.dma_start(tsum, ar_out[t].rearrange("u p z -> p u z"))
                    hn = psB.tile([P, H], F32, tag="hn")
                    for l in range(L):
                        e1 = sb3.tile([P, Z], F32, tag="e1")
                        nc.scalar.activation(e1, tsum[:, l], AF.Exp)
                        ssum = sb3.tile([P, 1], F32, tag="ssum")
                        nc.vector.reduce_sum(ssum, e1, mybir.AxisListType.X)
                        rec = sb3.tile([P, 1], F32, tag="rec")
                        nc.vector.reciprocal(rec, ssum)
                        s2 = sb3.tile([P, Z], F32, tag="s2")
                        nc.scalar.activation(s2, tsum[:, L + l], AF.Sigmoid)
                        sm = sb3.tile([P, Z], F32, tag="sm")
                        nc.vector.tensor_scalar_mul(sm, e1, rec)
                        comb = sb3.tile([P, Z], BF16, tag="comb")
                        nc.vector.tensor_mul(comb, s2, sm)
                        combT = sb3.tile([P, Z], BF16, tag="combT")
                        for k in range(KCH):
                            tpc = ps1.tile([P, P], BF16, tag="tp")
                            nc.tensor.transpose(tpc, comb[:, k * P:(k + 1) * P], ident)
                            nc.vector.tensor_copy(combT[:, k * P:(k + 1) * P], tpc)
                        for k in range(KCH):
                            nc.tensor.matmul(hn, combT[:, k * P:(k + 1) * P], wsl[:, l, k],
                                             start=(l == 0 and k == 0),
                                             stop=(l == L - 1 and k == KCH - 1))
                    hnew = sb3.tile([P, H], BF16, tag="hnew")
                    nc.vector.tensor_copy(hnew, hn)
                    for k in range(KCH):
                        tpd = ps1.tile([P, P], BF16, tag="tp")
                        nc.tensor.transpose(tpd, hnew[:, k * P:(k + 1) * P], ident)
                        nc.vector.tensor_copy(hgT[:, k * P:(k + 1) * P], tpd)
                    yp = ps1.tile([P, 1], F32, tag="tp")
                    for k in range(KCH):
                        nc.tensor.matmul(yp, hgT[:, k * P:(k + 1) * P],
                                         wlin[:, k, t:t + 1],
                                         start=(k == 0), stop=(k == KCH - 1))
                    nc.vector.tensor_copy(y_sb[:, t:t + 1], yp)
            nc.sync.dma_start(y_out[:], y_sb)
    nc.finalize()
    return nc


def _prep_inputs(x, Wx, Wh, Wg_h, Wg_p, Wilc, Wsl, Wlin):
    bf = ml_dtypes.bfloat16
    f8 = ml_dtypes.float8_e4m3
    s = W8SCALE
    xT = np.zeros((NC * TSH, KCH, P, B), dtype=bf)
    xT[:T] = np.ascontiguousarray(
        x.transpose(0, 2, 1).reshape(T, KCH, P, B)).astype(bf)
    wslT_full = np.zeros((NC * LSH, KCH, P, H), dtype=bf)
    for l in range(L):
        wslT_full[l] = Wsl[:, l * Z:(l + 1) * Z].T.reshape(KCH, P, H).astype(bf)
    in_maps = []
    for a in range(NC):
        w1 = np.empty((L, 2, KCH, P, 4 * H), dtype=f8)
        wg = np.empty((L, KCH, P, 3 * Z), dtype=f8)
        wgp = np.empty((L, KCH, P, 3 * Z), dtype=f8)
        wilc = np.empty((L, KCH, P, Z), dtype=f8)
        for l in range(L):
            w1[l, 0] = (Wx[a, l].transpose(2, 0, 1).reshape(KCH, P, 4 * H) * s).astype(f8)
            w1[l, 1] = (Wh[a, l].transpose(2, 0, 1).reshape(KCH, P, 4 * H) * s).astype(f8)
            wg[l] = (Wg_h[l, a].transpose(2, 0, 1).reshape(KCH, P, 3 * Z) * s).astype(f8)
            wgp[l] = (Wg_p[l, a].transpose(2, 0, 1).reshape(KCH, P, 3 * Z) * s).astype(f8)
            wilc[l] = (Wilc[l, a].reshape(KCH, P, Z) * s).astype(f8)
        wlinT = Wlin[:, 0, :].T.reshape(KCH, P, T).astype(bf)
        in_maps.append(dict(xs=np.ascontiguousarray(xT[a * TSH:(a + 1) * TSH]),
                            w1=w1, wg=wg, wgp=wgp, wilc=wilc,
                            wsls=np.ascontiguousarray(wslT_full[a * LSH:(a + 1) * LSH]),
                            wlinT=wlinT))
    return in_maps


def kernel(x, Wx, Wh, b_lstm, Wg_h, Wg_p, bg, Wilc, bilc, Wsl, bsl, Wlin, blin,
           _nsteps=T):
    x = np.asarray(x, np.float32)
    for nm, b in (("b_lstm", b_lstm), ("bg", bg), ("bilc", bilc), ("bsl", bsl),
                  ("blin", blin)):
        assert not np.any(np.asarray(b)), f"nonzero bias {nm} unsupported"
    in_maps = _prep_inputs(x, np.asarray(Wx, np.float32), np.asarray(Wh, np.float32),
                           np.asarray(Wg_h, np.float32), np.asarray(Wg_p, np.float32),
                           np.asarray(Wilc, np.float32), np.asarray(Wsl, np.float32),
                           np.asarray(Wlin, np.float32))
    nc = build(_nsteps)
    res = run_bass_kernel_spmd(nc, in_maps, list(range(NC)))
    y = np.asarray(res.results[0]["y"], np.float32)  # (B, T)
    return np.ascontiguousarray(y.T[:, :, None])  # (T, B, 1)


# revision 16
# speedup vs baseline: 9.8965x; 1.1023x over previous
"""Trainium2 Bass kernel for nn_D3MCELL (Multi-LSTM + cell_fn recurrence).

Axis-parallel sharding over 3 cores: core a in {0,1,2} runs axis a's stacked
LSTM and the axis-a part of cell_fn. The only cross-core traffic is one
AllReduce per timestep (t1/t2 partial sums over axes); the post-AR
softmax/combine/h_new is replicated on every core. bf16 matmuls, fp32 PSUM.

Big weights (Wx/Wh/Wg_h/Wg_p/Wilc) ship to the device as fp8e4 (scaled by
256) and are upcast to bf16 on-device in a prologue — host->device transfer
over the axon tunnel dominates wall time, not device compute.
"""
import os

os.environ.setdefault("JAX_COMPILATION_CACHE_DIR", "/root/.jax_cache")
os.environ.setdefault("JAX_PERSISTENT_CACHE_MIN_COMPILE_TIME_SECS", "0")
os.environ.setdefault("JAX_PERSISTENT_CACHE_MIN_ENTRY_SIZE_BYTES", "-1")
os.environ["BASS_DISABLE_FRAME_TO_TRACEBACK"] = "1"

import numpy as np
import ml_dtypes
import jax

try:
    jax.config.update("jax_compilation_cache_dir",
                      os.environ["JAX_COMPILATION_CACHE_DIR"])
    jax.config.update("jax_persistent_cache_min_compile_time_secs", 0)
    jax.config.update("jax_persistent_cache_min_entry_size_bytes", -1)
except Exception:
    pass

import concourse.bass as bass
import concourse.mybir as mybir
import concourse.tile as tile
from concourse import bacc
from concourse.bass import ds
from concourse.masks import make_identity
from concourse.bass_utils import run_bass_kernel_spmd

AF = mybir.ActivationFunctionType
BF16 = mybir.dt.bfloat16
FP8 = mybir.dt.float8e4
F32 = mybir.dt.float32

T, B, I, H, Z, A, L = 64, 128, 512, 512, 512, 3, 5
P = 128
NC = 3
KCH = H // P
W8SCALE = 256.0  # fp8 weights are pre-scaled by this on the host
TSH = (T + NC - 1) // NC   # 22: x timesteps per core (sharded, AllGathered)
LSH = (L + NC - 1) // NC   # 2: Wsl levels per core


def build(nsteps):
    nc = bacc.Bacc("TRN2", target_bir_lowering=False, debug=False,
                   num_devices=NC)
    xs_in = nc.declare_dram_parameter("xs", [TSH, KCH, P, P], BF16, isOutput=False)
    w1_in = nc.declare_dram_parameter("w1", [L, 2, KCH, P, 4 * H], FP8, isOutput=False)
    wg_in = nc.declare_dram_parameter("wg", [L, KCH, P, 3 * Z], FP8, isOutput=False)
    wgp_in = nc.declare_dram_parameter("wgp", [L, KCH, P, 3 * Z], FP8, isOutput=False)
    wilc_in = nc.declare_dram_parameter("wilc", [L, KCH, P, Z], FP8, isOutput=False)
    wsls_in = nc.declare_dram_parameter("wsls", [LSH, KCH, P, H], BF16, isOutput=False)
    wlin_in = nc.declare_dram_parameter("wlinT", [KCH, P, T], BF16, isOutput=False)
    y_out = nc.declare_dram_parameter("y", [P, T], F32, isOutput=True)
    xs_st = nc.dram_tensor("xs_st", [TSH, KCH, P, P], BF16)
    wsls_st = nc.dram_tensor("wsls_st", [LSH, KCH, P, H], BF16)
    xg = nc.dram_tensor("xg", [NC * TSH, KCH, P, P], BF16)
    wslg = nc.dram_tensor("wslg", [NC * LSH, KCH, P, H], BF16)

    h_hist = nc.dram_tensor("h_hist", [L, T, P, H], BF16)  # hT lhsT-ready
    c_hist = nc.dram_tensor("c_hist", [L, T, P, H], BF16)  # c batch-major
    ar_in = nc.dram_tensor("ar_in", [T, 2 * L, P, Z], BF16)
    ar_out = nc.dram_tensor("ar_out", [T, 2 * L, P, Z], BF16)
    w1d = nc.dram_tensor("w1d", [L, P, 2, KCH, 4 * H], BF16)   # upcast, pre-rearranged
    wgpd = nc.dram_tensor("wgpd", [L, P, KCH, 3 * Z], BF16)

    with tile.TileContext(nc) as tc:
        with (
            tc.tile_pool(name="const", bufs=1) as const,
            tc.tile_pool(name="state", bufs=1) as state,
            tc.tile_pool(name="wres", bufs=1) as wres,
            tc.tile_pool(name="ps1", bufs=2, space="PSUM") as ps1,
        ):
            ident = const.tile([P, P], BF16)
            make_identity(nc, ident)
            y_sb = state.tile([P, T], F32, tag="ysb")
            nc.vector.memset(y_sb, 0.0)
            hgT = state.tile([P, H], BF16, tag="hgT")
            nc.vector.memset(hgT, 0.0)
            wg = wres.tile([P, L, KCH, 3 * Z], BF16, tag="wg")
            wilc = wres.tile([P, L, KCH, Z], BF16, tag="wilc")
            wsl = wres.tile([P, L, KCH, H], BF16, tag="wsl")
            wlin = wres.tile([P, KCH, T], BF16, tag="wlin")

            # ------------- phase 0a: AllGather sharded x / Wsl -------------
            nc.sync.dma_start(xs_st[:], xs_in[:])
            nc.sync.dma_start(wsls_st[:], wsls_in[:])
            nc.gpsimd.collective_compute(
                "AllGather", mybir.AluOpType.bypass,
                ins=[xs_st[:]], outs=[xg[:]],
                replica_groups=[list(range(NC))],
            )
            nc.gpsimd.collective_compute(
                "AllGather", mybir.AluOpType.bypass,
                ins=[wsls_st[:]], outs=[wslg[:]],
                replica_groups=[list(range(NC))],
            )

            # ------------- phase 0: upcast fp8 weights to bf16 -------------
            with tc.tile_pool(name="up", bufs=1) as up:
                for l in range(L):
                    t8 = up.tile([P, 2, KCH, 4 * H], FP8, tag="t8")
                    nc.sync.dma_start(t8, w1_in[l].rearrange("s k p m -> p s k m"))
                    tb = up.tile([P, 2, KCH, 4 * H], BF16, tag="tb")
                    nc.vector.tensor_scalar_mul(tb, t8, 1.0 / W8SCALE)
                    nc.sync.dma_start(w1d[l], tb)
                for l in range(L):
                    g8 = up.tile([P, KCH, 3 * Z], FP8, tag="g8")
                    nc.sync.dma_start(g8, wgp_in[l].rearrange("k p m -> p k m"))
                    gb = up.tile([P, KCH, 3 * Z], BF16, tag="gb")
                    nc.vector.tensor_scalar_mul(gb, g8, 1.0 / W8SCALE)
                    nc.sync.dma_start(wgpd[l], gb)
                    h8 = up.tile([P, KCH, 3 * Z], FP8, tag="g8")
                    nc.sync.dma_start(h8, wg_in[l].rearrange("k p m -> p k m"))
                    nc.vector.tensor_scalar_mul(wg[:, l], h8, 1.0 / W8SCALE)
                i8 = up.tile([P, L, KCH, Z], FP8, tag="i8")
                nc.sync.dma_start(i8, wilc_in[:].rearrange("l k p m -> p l k m"))
                nc.vector.tensor_scalar_mul(wilc, i8, 1.0 / W8SCALE)
                for l in range(L):
                    nc.sync.dma_start(wsl[:, l], wslg[l].rearrange("k p m -> p k m"))
                nc.sync.dma_start(wlin, wlin_in[:].rearrange("k p t -> p k t"))

            # ------------- phase 1: stacked LSTM, level-serial -------------
            with (
                tc.tile_pool(name="wp1", bufs=1) as wp1,
                tc.tile_pool(name="st1", bufs=1) as st1,
                tc.tile_pool(name="sb1", bufs=2) as sb1,
                tc.tile_pool(name="psA", bufs=1, space="PSUM") as psA,
            ):
                xg_v = xg.rearrange("t k p m -> p t k m")
                for l in range(L):
                    w1l = wp1.tile([P, 2, KCH, 4 * H], BF16, tag="w1lev")
                    nc.sync.dma_start(w1l, w1d[l])
                    hT = st1.tile([P, H], BF16, tag=f"hT_{l}")
                    nc.vector.memset(hT, 0.0)
                    cst = st1.tile([P, H], F32, tag=f"c_{l}")
                    nc.vector.memset(cst, 0.0)
                    if l > 0:
                        hsrc_v = h_hist[l - 1].rearrange("t p (k m) -> p t k m", k=KCH)
                    hdst_v = h_hist[l].rearrange("t p h -> p t h")
                    cdst_v = c_hist[l].rearrange("t p h -> p t h")
                    with tc.For_i(0, nsteps) as t:
                        pg4 = [psA.tile([P, 512], F32, tag=f"pg_{g}", name=f"pg_{g}") for g in range(4)]
                        inpT = sb1.tile([P, 1, KCH, P], BF16, tag="inpT")
                        if l == 0:
                            nc.sync.dma_start(inpT, xg_v[:, ds(t, 1)])
                        else:
                            nc.sync.dma_start(inpT, hsrc_v[:, ds(t, 1)])
                        for k in range(KCH):
                            for g in range(4):
                                nc.tensor.matmul(pg4[g], inpT[:, 0, k],
                                                 w1l[:, 0, k, g * 512:(g + 1) * 512],
                                                 start=(k == 0), stop=False)
                        for k in range(KCH):
                            hk = hT[:, k * P:(k + 1) * P]
                            for g in range(4):
                                nc.tensor.matmul(pg4[g], hk,
                                                 w1l[:, 1, k, g * 512:(g + 1) * 512],
                                                 start=False, stop=(k == KCH - 1))
                        si = sb1.tile([P, 512], F32, tag="si")
                        nc.scalar.activation(si, pg4[0], AF.Sigmoid)
                        sf = sb1.tile([P, 512], F32, tag="sf")
                        nc.scalar.activation(sf, pg4[1], AF.Sigmoid)
                        tg = sb1.tile([P, 512], F32, tag="tg")
                        nc.scalar.activation(tg, pg4[2], AF.Tanh)
                        so = sb1.tile([P, 512], F32, tag="so")
                        nc.scalar.activation(so, pg4[3], AF.Sigmoid)
                        fc = sb1.tile([P, 512], F32, tag="fc")
                        nc.vector.tensor_mul(fc, sf, cst)
                        ig = sb1.tile([P, 512], F32, tag="ig")
                        nc.vector.tensor_mul(ig, si, tg)
                        nc.vector.tensor_add(cst, fc, ig)
                        tch = sb1.tile([P, 512], F32, tag="tch")
                        nc.scalar.activation(tch, cst, AF.Tanh)
                        h_bf = sb1.tile([P, 512], BF16, tag="h_bf")
                        nc.vector.tensor_mul(h_bf, so, tch)
                        c_bf = sb1.tile([P, 512], BF16, tag="c_bf")
                        nc.vector.tensor_copy(c_bf, cst)
                        nc.sync.dma_start(cdst_v[:, ds(t, 1)],
                                          c_bf.rearrange("p (o h) -> p o h", o=1))
                        for k in range(KCH):
                            tp = ps1.tile([P, P], BF16, tag="tp")
                            nc.tensor.transpose(tp, h_bf[:, k * P:(k + 1) * P], ident)
                            nc.vector.tensor_copy(hT[:, k * P:(k + 1) * P], tp)
                        nc.sync.dma_start(hdst_v[:, ds(t, 1)],
                                          hT.rearrange("p (o h) -> p o h", o=1))

            # ------------- phase 3: cell_fn recurrence -------------
            with (
                tc.tile_pool(name="wgpp", bufs=2) as wgpp,
                tc.tile_pool(name="tpool", bufs=1) as tpool,
                tc.tile_pool(name="sb3", bufs=2) as sb3,
                tc.tile_pool(name="psB", bufs=1, space="PSUM") as psB,
            ):
                for t in range(nsteps):
                    tparts = tpool.tile([P, 2 * L, Z], BF16, tag="tparts")
                    for l in range(L):
                        pr3 = [psB.tile([P, 512], F32, tag=f"pr_{g}", name=f"pr_{g}") for g in range(3)]
                        shT = sb3.tile([P, H], BF16, tag="shT")
                        nc.sync.dma_start(shT, h_hist[l, t])
                        wgpl = wgpp.tile([P, KCH, 3 * Z], BF16, tag="wgpl")
                        nc.sync.dma_start(wgpl, wgpd[l])
                        for k in range(KCH):
                            for g in range(3):
                                nc.tensor.matmul(pr3[g], shT[:, k * P:(k + 1) * P],
                                                 wgpl[:, k, g * 512:(g + 1) * 512],
                                                 start=(k == 0), stop=False)
                        for k in range(KCH):
                            for g in range(3):
                                nc.tensor.matmul(pr3[g], hgT[:, k * P:(k + 1) * P],
                                                 wg[:, l, k, g * 512:(g + 1) * 512],
                                                 start=False, stop=(k == KCH - 1))
                        si3 = sb3.tile([P, 512], F32, tag="si3")
                        nc.scalar.activation(si3, pr3[0], AF.Sigmoid)
                        sf3 = sb3.tile([P, 512], F32, tag="sf3")
                        nc.scalar.activation(sf3, pr3[1], AF.Sigmoid)
                        tg3 = sb3.tile([P, 512], F32, tag="tg3")
                        nc.scalar.activation(tg3, pr3[2], AF.Tanh)
                        scc = sb3.tile([P, 512], BF16, tag="scc")
                        nc.sync.dma_start(scc, c_hist[l, t])
                        icell = sb3.tile([P, 512], BF16, tag="icell")
                        nc.vector.tensor_mul(icell, si3, scc)
                        fg = sb3.tile([P, 512], F32, tag="fg")
                        nc.vector.tensor_mul(fg, sf3, tg3)
                        ccell = sb3.tile([P, 512], BF16, tag="ccell")
                        nc.vector.tensor_add(ccell, fg, icell)
                        icT = sb3.tile([P, 512], BF16, tag="icT")
                        ccT = sb3.tile([P, 512], BF16, tag="ccT")
                        for k in range(KCH):
                            tpa = ps1.tile([P, P], BF16, tag="tp")
                            nc.tensor.transpose(tpa, icell[:, k * P:(k + 1) * P], ident)
                            nc.vector.tensor_copy(icT[:, k * P:(k + 1) * P], tpa)
                            tpb = ps1.tile([P, P], BF16, tag="tp")
                            nc.tensor.transpose(tpb, ccell[:, k * P:(k + 1) * P], ident)
                            nc.vector.tensor_copy(ccT[:, k * P:(k + 1) * P], tpb)
                        t1p = psB.tile([P, Z], F32, tag="t1p")
                        t2p = psB.tile([P, Z], F32, tag="t2p")
                        for k in range(KCH):
                            nc.tensor.matmul(t1p, icT[:, k * P:(k + 1) * P], wilc[:, l, k],
                                             start=(k == 0), stop=(k == KCH - 1))
                        for k in range(KCH):
                            nc.tensor.matmul(t2p, ccT[:, k * P:(k + 1) * P], wilc[:, l, k],
                                             start=(k == 0), stop=(k == KCH - 1))
                        nc.vector.tensor_copy(tparts[:, l], t1p)
                        nc.vector.tensor_copy(tparts[:, L + l], t2p)
                    nc.sync.dma_start(ar_in[t].rearrange("u p z -> p u z"), tparts)
                    nc.gpsimd.collective_compute(
                        "AllReduce", mybir.AluOpType.add,
                        ins=[ar_in[t]], outs=[ar_out[t]],
                        replica_groups=[list(range(NC))],
                    )
                    tsum = tpool.tile([P, 2 * L, Z], BF16, tag="tsum")
                    nc.sync.dma_start(tsum, ar_out[t].rearrange("u p z -> p u z"))
                    hn = psB.tile([P, H], F32, tag="hn")
                    for l in range(L):
                        e1 = sb3.tile([P, Z], F32, tag="e1")
                        nc.scalar.activation(e1, tsum[:, l], AF.Exp)
                        ssum = sb3.tile([P, 1], F32, tag="ssum")
                        nc.vector.reduce_sum(ssum, e1, mybir.AxisListType.X)
                        rec = sb3.tile([P, 1], F32, tag="rec")
                        nc.vector.reciprocal(rec, ssum)
                        s2 = sb3.tile([P, Z], F32, tag="s2")
                        nc.scalar.activation(s2, tsum[:, L + l], AF.Sigmoid)
                        sm = sb3.tile([P, Z], F32, tag="sm")
                        nc.vector.tensor_scalar_mul(sm, e1, rec)
                        comb = sb3.tile([P, Z], BF16, tag="comb")
                        nc.vector.tensor_mul(comb, s2, sm)
                        combT = sb3.tile([P, Z], BF16, tag="combT")
                        for k in range(KCH):
                            tpc = ps1.tile([P, P], BF16, tag="tp")
                            nc.tensor.transpose(tpc, comb[:, k * P:(k + 1) * P], ident)
                            nc.vector.tensor_copy(combT[:, k * P:(k + 1) * P], tpc)
                        for k in range(KCH):
                            nc.tensor.matmul(hn, combT[:, k * P:(k + 1) * P], wsl[:, l, k],
                                             start=(l == 0 and k == 0),
                                             stop=(l == L - 1 and k == KCH - 1))
                    hnew = sb3.tile([P, H], BF16, tag="hnew")
                    nc.vector.tensor_copy(hnew, hn)
                    for k in range(KCH):
                        tpd = ps1.tile([P, P], BF16, tag="tp")
                        nc.tensor.transpose(tpd, hnew[:, k * P:(k + 1) * P], ident)
                        nc.vector.tensor_copy(hgT[:, k * P:(k + 1) * P], tpd)
                    yp = ps1.tile([P, 1], F32, tag="tp")
                    for k in range(KCH):
                        nc.tensor.matmul(yp, hgT[:, k * P:(k + 1) * P],
                                         wlin[:, k, t:t + 1],
                                         start=(k == 0), stop=(k == KCH - 1))
                    nc.vector.tensor_copy(y_sb[:, t:t + 1], yp)
            nc.sync.dma_start(y_out[:], y_sb)
    nc.finalize()
    return nc


def _prep_inputs(x, Wx, Wh, Wg_h, Wg_p, Wilc, Wsl, Wlin):
    bf = ml_dtypes.bfloat16
    f8 = ml_dtypes.float8_e4m3
    s = W8SCALE
    xT = np.zeros((NC * TSH, KCH, P, B), dtype=bf)
    xT[:T] = np.ascontiguousarray(
        x.transpose(0, 2, 1).reshape(T, KCH, P, B)).astype(bf)
    wslT_full = np.zeros((NC * LSH, KCH, P, H), dtype=bf)
    for l in range(L):
        wslT_full[l] = Wsl[:, l * Z:(l + 1) * Z].T.reshape(KCH, P, H).astype(bf)
    in_maps = []
    for a in range(NC):
        w1 = np.empty((L, 2, KCH, P, 4 * H), dtype=f8)
        wg = np.empty((L, KCH, P, 3 * Z), dtype=f8)
        wgp = np.empty((L, KCH, P, 3 * Z), dtype=f8)
        wilc = np.empty((L, KCH, P, Z), dtype=f8)
        for l in range(L):
            w1[l, 0] = (Wx[a, l].transpose(2, 0, 1).reshape(KCH, P, 4 * H) * s).astype(f8)
            w1[l, 1] = (Wh[a, l].transpose(2, 0, 1).reshape(KCH, P, 4 * H) * s).astype(f8)
            wg[l] = (Wg_h[l, a].transpose(2, 0, 1).reshape(KCH, P, 3 * Z) * s).astype(f8)
            wgp[l] = (Wg_p[l, a].transpose(2, 0, 1).reshape(KCH, P, 3 * Z) * s).astype(f8)
            wilc[l] = (Wilc[l, a].reshape(KCH, P, Z) * s).astype(f8)
        wlinT = Wlin[:, 0, :].T.reshape(KCH, P, T).astype(bf)
        in_maps.append(dict(xs=np.ascontiguousarray(xT[a * TSH:(a + 1) * TSH]),
                            w1=w1, wg=wg, wgp=wgp, wilc=wilc,
                            wsls=np.ascontiguousarray(wslT_full[a * LSH:(a + 1) * LSH]),
                            wlinT=wlinT))
    return in_maps


def kernel(x, Wx, Wh, b_lstm, Wg_h, Wg_p, bg, Wilc, bilc, Wsl, bsl, Wlin, blin,
           _nsteps=T):
    x = np.asarray(x, np.float32)
    for nm, b in (("b_lstm", b_lstm), ("bg", bg), ("bilc", bilc), ("bsl", bsl),
                  ("blin", blin)):
        assert not np.any(np.asarray(b)), f"nonzero bias {nm} unsupported"
    in_maps = _prep_inputs(x, np.asarray(Wx, np.float32), np.asarray(Wh, np.float32),
                           np.asarray(Wg_h, np.float32), np.asarray(Wg_p, np.float32),
                           np.asarray(Wilc, np.float32), np.asarray(Wsl, np.float32),
                           np.asarray(Wlin, np.float32))
    nc = build(_nsteps)
    res = run_bass_kernel_spmd(nc, in_maps, list(range(NC)))
    y = np.asarray(res.results[0]["y"], np.float32)  # (B, T)
    return np.ascontiguousarray(y.T[:, :, None])  # (T, B, 1)


# revision 19
# speedup vs baseline: 10.8002x; 1.0913x over previous
"""Trainium2 Bass kernel for nn_D3MCELL (Multi-LSTM + cell_fn recurrence).

Axis-parallel sharding over 3 cores: core a in {0,1,2} runs axis a's stacked
LSTM and the axis-a part of cell_fn. The only cross-core traffic is one
AllReduce per timestep (t1/t2 partial sums over axes); the post-AR
softmax/combine/h_new is replicated on every core. bf16 matmuls, fp32 PSUM.

Big weights (Wx/Wh/Wg_h/Wg_p/Wilc) ship to the device as fp8e4 (scaled by
256) and are upcast to bf16 on-device in a prologue — host->device transfer
over the axon tunnel dominates wall time, not device compute.
"""
import os

os.environ.setdefault("JAX_COMPILATION_CACHE_DIR", "/root/.jax_cache")
os.environ.setdefault("JAX_PERSISTENT_CACHE_MIN_COMPILE_TIME_SECS", "0")
os.environ.setdefault("JAX_PERSISTENT_CACHE_MIN_ENTRY_SIZE_BYTES", "-1")
os.environ["BASS_DISABLE_FRAME_TO_TRACEBACK"] = "1"

import numpy as np
import ml_dtypes
import jax

try:
    jax.config.update("jax_compilation_cache_dir",
                      os.environ["JAX_COMPILATION_CACHE_DIR"])
    jax.config.update("jax_persistent_cache_min_compile_time_secs", 0)
    jax.config.update("jax_persistent_cache_min_entry_size_bytes", -1)
except Exception:
    pass

import concourse.bass as bass
import concourse.mybir as mybir
import concourse.tile as tile
from concourse import bacc
from concourse.bass import ds
from concourse.masks import make_identity
from concourse.bass_utils import run_bass_kernel_spmd

AF = mybir.ActivationFunctionType
BF16 = mybir.dt.bfloat16
FP8 = mybir.dt.float8e4
F32 = mybir.dt.float32

T, B, I, H, Z, A, L = 64, 128, 512, 512, 512, 3, 5
P = 128
NC = 3
KCH = H // P
W8SCALE = 256.0  # fp8 weights are pre-scaled by this on the host
TSH = (T + NC - 1) // NC   # 22: x timesteps per core (sharded, AllGathered)
LSH = (L + NC - 1) // NC   # 2: Wsl levels per core


def build(nsteps):
    nc = bacc.Bacc("TRN2", target_bir_lowering=False, debug=False,
                   num_devices=NC)
    xs_in = nc.declare_dram_parameter("xs", [TSH, KCH, P, P], BF16, isOutput=False)
    w1_in = nc.declare_dram_parameter("w1", [L, 2, KCH, P, 4 * H], FP8, isOutput=False)
    wg_in = nc.declare_dram_parameter("wg", [L, KCH, P, 3 * Z], FP8, isOutput=False)
    wgp_in = nc.declare_dram_parameter("wgp", [L, KCH, P, 3 * Z], FP8, isOutput=False)
    wilc_in = nc.declare_dram_parameter("wilc", [L, KCH, P, Z], FP8, isOutput=False)
    wsls_in = nc.declare_dram_parameter("wsls", [LSH, KCH, P, H], BF16, isOutput=False)
    wlin_in = nc.declare_dram_parameter("wlinT", [KCH, P, T], BF16, isOutput=False)
    y_out = nc.declare_dram_parameter("y", [P, T], F32, isOutput=True)
    xs_st = nc.dram_tensor("xs_st", [TSH, KCH, P, P], BF16)
    wsls_st = nc.dram_tensor("wsls_st", [LSH, KCH, P, H], BF16)
    xg = nc.dram_tensor("xg", [NC * TSH, KCH, P, P], BF16)
    wslg = nc.dram_tensor("wslg", [NC * LSH, KCH, P, H], BF16)

    h_hist = nc.dram_tensor("h_hist", [L, T, P, H], BF16)  # hT lhsT-ready
    c_hist = nc.dram_tensor("c_hist", [L, T, P, H], BF16)  # c batch-major
    ar_in = nc.dram_tensor("ar_in", [T, 2 * L, P, Z], BF16)
    ar_out = nc.dram_tensor("ar_out", [T, 2 * L, P, Z], BF16)
    w1d = nc.dram_tensor("w1d", [L, P, 2, KCH, 4 * H], BF16)   # upcast, pre-rearranged
    wgpd = nc.dram_tensor("wgpd", [L, P, KCH, 3 * Z], BF16)
    pre_p = nc.dram_tensor("pre_p", [L, T, P, 3 * Z], BF16)    # sh @ Wg_p, all (l,t)

    with tile.TileContext(nc) as tc:
        with (
            tc.tile_pool(name="const", bufs=1) as const,
            tc.tile_pool(name="state", bufs=1) as state,
            tc.tile_pool(name="wres", bufs=1) as wres,
            tc.tile_pool(name="ps1", bufs=2, space="PSUM") as ps1,
        ):
            ident = const.tile([P, P], BF16)
            make_identity(nc, ident)
            y_sb = state.tile([P, T], F32, tag="ysb")
            nc.vector.memset(y_sb, 0.0)
            hgT = state.tile([P, H], BF16, tag="hgT")
            nc.vector.memset(hgT, 0.0)
            wg = wres.tile([P, L, KCH, 3 * Z], BF16, tag="wg")
            wilc = wres.tile([P, L, KCH, Z], BF16, tag="wilc")
            wsl = wres.tile([P, L, KCH, H], BF16, tag="wsl")
            wlin = wres.tile([P, KCH, T], BF16, tag="wlin")

            # ------------- phase 0a: AllGather sharded x / Wsl -------------
            nc.sync.dma_start(xs_st[:], xs_in[:])
            nc.sync.dma_start(wsls_st[:], wsls_in[:])
            nc.gpsimd.collective_compute(
                "AllGather", mybir.AluOpType.bypass,
                ins=[xs_st[:]], outs=[xg[:]],
                replica_groups=[list(range(NC))],
            )
            nc.gpsimd.collective_compute(
                "AllGather", mybir.AluOpType.bypass,
                ins=[wsls_st[:]], outs=[wslg[:]],
                replica_groups=[list(range(NC))],
            )

            # ------------- phase 0: upcast fp8 weights to bf16 -------------
            with tc.tile_pool(name="up", bufs=1) as up:
                for l in range(L):
                    t8 = up.tile([P, 2, KCH, 4 * H], FP8, tag="t8")
                    nc.sync.dma_start(t8, w1_in[l].rearrange("s k p m -> p s k m"))
                    tb = up.tile([P, 2, KCH, 4 * H], BF16, tag="tb")
                    nc.vector.tensor_scalar_mul(tb, t8, 1.0 / W8SCALE)
                    nc.sync.dma_start(w1d[l], tb)
                for l in range(L):
                    g8 = up.tile([P, KCH, 3 * Z], FP8, tag="g8")
                    nc.sync.dma_start(g8, wgp_in[l].rearrange("k p m -> p k m"))
                    gb = up.tile([P, KCH, 3 * Z], BF16, tag="gb")
                    nc.vector.tensor_scalar_mul(gb, g8, 1.0 / W8SCALE)
                    nc.sync.dma_start(wgpd[l], gb)
                    h8 = up.tile([P, KCH, 3 * Z], FP8, tag="g8")
                    nc.sync.dma_start(h8, wg_in[l].rearrange("k p m -> p k m"))
                    nc.vector.tensor_scalar_mul(wg[:, l], h8, 1.0 / W8SCALE)
                i8 = up.tile([P, L, KCH, Z], FP8, tag="i8")
                nc.sync.dma_start(i8, wilc_in[:].rearrange("l k p m -> p l k m"))
                nc.vector.tensor_scalar_mul(wilc, i8, 1.0 / W8SCALE)
                for l in range(L):
                    nc.sync.dma_start(wsl[:, l], wslg[l].rearrange("k p m -> p k m"))
                nc.sync.dma_start(wlin, wlin_in[:].rearrange("k p t -> p k t"))

            # ------------- phase 1: stacked LSTM, level-serial -------------
            with (
                tc.tile_pool(name="wp1", bufs=1) as wp1,
                tc.tile_pool(name="st1", bufs=1) as st1,
                tc.tile_pool(name="sb1", bufs=2) as sb1,
                tc.tile_pool(name="psA", bufs=1, space="PSUM") as psA,
            ):
                xg_v = xg.rearrange("t k p m -> p t k m")
                for l in range(L):
                    w1l = wp1.tile([P, 2, KCH, 4 * H], BF16, tag="w1lev")
                    nc.sync.dma_start(w1l, w1d[l])
                    hT = st1.tile([P, H], BF16, tag=f"hT_{l}")
                    nc.vector.memset(hT, 0.0)
                    cst = st1.tile([P, H], F32, tag=f"c_{l}")
                    nc.vector.memset(cst, 0.0)
                    if l > 0:
                        hsrc_v = h_hist[l - 1].rearrange("t p (k m) -> p t k m", k=KCH)
                    hdst_v = h_hist[l].rearrange("t p h -> p t h")
                    cdst_v = c_hist[l].rearrange("t p h -> p t h")
                    with tc.For_i(0, nsteps) as t:
                        pg4 = [psA.tile([P, 512], F32, tag=f"pg_{g}", name=f"pg_{g}") for g in range(4)]
                        inpT = sb1.tile([P, 1, KCH, P], BF16, tag="inpT")
                        if l == 0:
                            nc.sync.dma_start(inpT, xg_v[:, ds(t, 1)])
                        else:
                            nc.sync.dma_start(inpT, hsrc_v[:, ds(t, 1)])
                        for k in range(KCH):
                            for g in range(4):
                                nc.tensor.matmul(pg4[g], inpT[:, 0, k],
                                                 w1l[:, 0, k, g * 512:(g + 1) * 512],
                                                 start=(k == 0), stop=False)
                        for k in range(KCH):
                            hk = hT[:, k * P:(k + 1) * P]
                            for g in range(4):
                                nc.tensor.matmul(pg4[g], hk,
                                                 w1l[:, 1, k, g * 512:(g + 1) * 512],
                                                 start=False, stop=(k == KCH - 1))
                        si = sb1.tile([P, 512], F32, tag="si")
                        nc.scalar.activation(si, pg4[0], AF.Sigmoid)
                        sf = sb1.tile([P, 512], F32, tag="sf")
                        nc.scalar.activation(sf, pg4[1], AF.Sigmoid)
                        tg = sb1.tile([P, 512], F32, tag="tg")
                        nc.scalar.activation(tg, pg4[2], AF.Tanh)
                        so = sb1.tile([P, 512], F32, tag="so")
                        nc.scalar.activation(so, pg4[3], AF.Sigmoid)
                        fc = sb1.tile([P, 512], F32, tag="fc")
                        nc.vector.tensor_mul(fc, sf, cst)
                        ig = sb1.tile([P, 512], F32, tag="ig")
                        nc.vector.tensor_mul(ig, si, tg)
                        nc.vector.tensor_add(cst, fc, ig)
                        tch = sb1.tile([P, 512], F32, tag="tch")
                        nc.scalar.activation(tch, cst, AF.Tanh)
                        h_bf = sb1.tile([P, 512], BF16, tag="h_bf")
                        nc.vector.tensor_mul(h_bf, so, tch)
                        c_bf = sb1.tile([P, 512], BF16, tag="c_bf")
                        nc.vector.tensor_copy(c_bf, cst)
                        nc.sync.dma_start(cdst_v[:, ds(t, 1)],
                                          c_bf.rearrange("p (o h) -> p o h", o=1))
                        for k in range(KCH):
                            tp = ps1.tile([P, P], BF16, tag="tp")
                            nc.tensor.transpose(tp, h_bf[:, k * P:(k + 1) * P], ident)
                            nc.vector.tensor_copy(hT[:, k * P:(k + 1) * P], tp)
                        nc.sync.dma_start(hdst_v[:, ds(t, 1)],
                                          hT.rearrange("p (o h) -> p o h", o=1))

            # ------- phase 2.5: pre_p[l,t] = sh[l,t] @ Wg_p[l], For_i over t -------
            with (
                tc.tile_pool(name="wp2", bufs=1) as wp2,
                tc.tile_pool(name="sb2", bufs=2) as sb2,
                tc.tile_pool(name="ps2", bufs=1, space="PSUM") as ps2,
            ):
                wgp_all = wp2.tile([P, L, KCH, 3 * Z], BF16, tag="wgp_all")
                nc.sync.dma_start(wgp_all, wgpd.rearrange("l p k m -> p l k m"))
                hviews = [h_hist[l].rearrange("t p (k m) -> p t k m", k=KCH)
                          for l in range(L)]
                pviews = [pre_p[l].rearrange("t p m -> p t m") for l in range(L)]
                with tc.For_i(0, nsteps) as t:
                    for l in range(L):
                        sh_t = sb2.tile([P, 1, KCH, P], BF16, tag="sh_t")
                        nc.sync.dma_start(sh_t, hviews[l][:, ds(t, 1)])
                        pr = [ps2.tile([P, 512], F32, tag=f"pp_{g}", name=f"pp_{g}") for g in range(3)]
                        for k in range(KCH):
                            for g in range(3):
                                nc.tensor.matmul(pr[g], sh_t[:, 0, k],
                                                 wgp_all[:, l, k, g * 512:(g + 1) * 512],
                                                 start=(k == 0), stop=(k == KCH - 1))
                        ppt = sb2.tile([P, 1, 3 * Z], BF16, tag="ppt")
                        for g in range(3):
                            nc.vector.tensor_copy(ppt[:, 0, g * 512:(g + 1) * 512], pr[g])
                        nc.sync.dma_start(pviews[l][:, ds(t, 1)], ppt)

            # ------------- phase 3: cell_fn recurrence -------------
            with (
                tc.tile_pool(name="tpool", bufs=1) as tpool,
                tc.tile_pool(name="sb3", bufs=2) as sb3,
                tc.tile_pool(name="psB", bufs=1, space="PSUM") as psB,
            ):
                for t in range(nsteps):
                    tparts = tpool.tile([P, 2 * L, Z], BF16, tag="tparts")
                    for l in range(L):
                        pr3 = [psB.tile([P, 512], F32, tag=f"pr_{g}", name=f"pr_{g}") for g in range(3)]
                        ppl = sb3.tile([P, 3 * Z], BF16, tag="ppl")
                        nc.sync.dma_start(ppl, pre_p[l, t])
                        for k in range(KCH):
                            for g in range(3):
                                nc.tensor.matmul(pr3[g], hgT[:, k * P:(k + 1) * P],
                                                 wg[:, l, k, g * 512:(g + 1) * 512],
                                                 start=(k == 0), stop=(k == KCH - 1))
                        pa = [sb3.tile([P, 512], F32, tag=f"pa_{g}", name=f"pa_{g}") for g in range(3)]
                        for g in range(3):
                            nc.vector.tensor_add(pa[g], pr3[g],
                                                 ppl[:, g * 512:(g + 1) * 512])
                        si3 = sb3.tile([P, 512], F32, tag="si3")
                        nc.scalar.activation(si3, pa[0], AF.Sigmoid)
                        sf3 = sb3.tile([P, 512], F32, tag="sf3")
                        nc.scalar.activation(sf3, pa[1], AF.Sigmoid)
                        tg3 = sb3.tile([P, 512], F32, tag="tg3")
                        nc.scalar.activation(tg3, pa[2], AF.Tanh)
                        scc = sb3.tile([P, 512], BF16, tag="scc")
                        nc.sync.dma_start(scc, c_hist[l, t])
                        icell = sb3.tile([P, 512], BF16, tag="icell")
                        nc.vector.tensor_mul(icell, si3, scc)
                        fg = sb3.tile([P, 512], F32, tag="fg")
                        nc.vector.tensor_mul(fg, sf3, tg3)
                        ccell = sb3.tile([P, 512], BF16, tag="ccell")
                        nc.vector.tensor_add(ccell, fg, icell)
                        icT = sb3.tile([P, 512], BF16, tag="icT")
                        ccT = sb3.tile([P, 512], BF16, tag="ccT")
                        for k in range(KCH):
                            tpa = ps1.tile([P, P], BF16, tag="tp")
                            nc.tensor.transpose(tpa, icell[:, k * P:(k + 1) * P], ident)
                            nc.vector.tensor_copy(icT[:, k * P:(k + 1) * P], tpa)
                            tpb = ps1.tile([P, P], BF16, tag="tp")
                            nc.tensor.transpose(tpb, ccell[:, k * P:(k + 1) * P], ident)
                            nc.vector.tensor_copy(ccT[:, k * P:(k + 1) * P], tpb)
                        t1p = psB.tile([P, Z], F32, tag="t1p")
                        t2p = psB.tile([P, Z], F32, tag="t2p")
                        for k in range(KCH):
                            nc.tensor.matmul(t1p, icT[:, k * P:(k + 1) * P], wilc[:, l, k],
                                             start=(k == 0), stop=(k == KCH - 1))
                        for k in range(KCH):
                            nc.tensor.matmul(t2p, ccT[:, k * P:(k + 1) * P], wilc[:, l, k],
                                             start=(k == 0), stop=(k == KCH - 1))
                        nc.vector.tensor_copy(tparts[:, l], t1p)
                        nc.vector.tensor_copy(tparts[:, L + l], t2p)
                    nc.sync.dma_start(ar_in[t].rearrange("u p z -> p u z"), tparts)
                    nc.gpsimd.collective_compute(
                        "AllReduce", mybir.AluOpType.add,
                        ins=[ar_in[t]], outs=[ar_out[t]],
                        replica_groups=[list(range(NC))],
                    )
                    tsum = tpool.tile([P, 2 * L, Z], BF16, tag="tsum")
                    nc.sync.dma_start(tsum, ar_out[t].rearrange("u p z -> p u z"))
                    hn = psB.tile([P, H], F32, tag="hn")
                    for l in range(L):
                        e1 = sb3.tile([P, Z], F32, tag="e1")
                        nc.scalar.activation(e1, tsum[:, l], AF.Exp)
                        ssum = sb3.tile([P, 1], F32, tag="ssum")
                        nc.vector.reduce_sum(ssum, e1, mybir.AxisListType.X)
                        rec = sb3.tile([P, 1], F32, tag="rec")
                        nc.vector.reciprocal(rec, ssum)
                        s2 = sb3.tile([P, Z], F32, tag="s2")
                        nc.scalar.activation(s2, tsum[:, L + l], AF.Sigmoid)
                        sm = sb3.tile([P, Z], F32, tag="sm")
                        nc.vector.tensor_scalar_mul(sm, e1, rec)
                        comb = sb3.tile([P, Z], BF16, tag="comb")
                        nc.vector.tensor_mul(comb, s2, sm)
                        combT = sb3.tile([P, Z], BF16, tag="combT")
                        for k in range(KCH):
                            tpc = ps1.tile([P, P], BF16, tag="tp")
                            nc.tensor.transpose(tpc, comb[:, k * P:(k + 1) * P], ident)
                            nc.vector.tensor_copy(combT[:, k * P:(k + 1) * P], tpc)
                        for k in range(KCH):
                            nc.tensor.matmul(hn, combT[:, k * P:(k + 1) * P], wsl[:, l, k],
                                             start=(l == 0 and k == 0),
                                             stop=(l == L - 1 and k == KCH - 1))
                    hnew = sb3.tile([P, H], BF16, tag="hnew")
                    nc.vector.tensor_copy(hnew, hn)
                    for k in range(KCH):
                        tpd = ps1.tile([P, P], BF16, tag="tp")
                        nc.tensor.transpose(tpd, hnew[:, k * P:(k + 1) * P], ident)
                        nc.vector.tensor_copy(hgT[:, k * P:(k + 1) * P], tpd)
                    yp = ps1.tile([P, 1], F32, tag="tp")
                    for k in range(KCH):
                        nc.tensor.matmul(yp, hgT[:, k * P:(k + 1) * P],
                                         wlin[:, k, t:t + 1],
                                         start=(k == 0), stop=(k == KCH - 1))
                    nc.vector.tensor_copy(y_sb[:, t:t + 1], yp)
            nc.sync.dma_start(y_out[:], y_sb)
    nc.finalize()
    return nc


def _prep_inputs(x, Wx, Wh, Wg_h, Wg_p, Wilc, Wsl, Wlin):
    bf = ml_dtypes.bfloat16
    f8 = ml_dtypes.float8_e4m3
    s = W8SCALE
    xT = np.zeros((NC * TSH, KCH, P, B), dtype=bf)
    xT[:T] = np.ascontiguousarray(
        x.transpose(0, 2, 1).reshape(T, KCH, P, B)).astype(bf)
    wslT_full = np.zeros((NC * LSH, KCH, P, H), dtype=bf)
    for l in range(L):
        wslT_full[l] = Wsl[:, l * Z:(l + 1) * Z].T.reshape(KCH, P, H).astype(bf)
    in_maps = []
    for a in range(NC):
        w1 = np.empty((L, 2, KCH, P, 4 * H), dtype=f8)
        wg = np.empty((L, KCH, P, 3 * Z), dtype=f8)
        wgp = np.empty((L, KCH, P, 3 * Z), dtype=f8)
        wilc = np.empty((L, KCH, P, Z), dtype=f8)
        for l in range(L):
            w1[l, 0] = (Wx[a, l].transpose(2, 0, 1).reshape(KCH, P, 4 * H) * s).astype(f8)
            w1[l, 1] = (Wh[a, l].transpose(2, 0, 1).reshape(KCH, P, 4 * H) * s).astype(f8)
            wg[l] = (Wg_h[l, a].transpose(2, 0, 1).reshape(KCH, P, 3 * Z) * s).astype(f8)
            wgp[l] = (Wg_p[l, a].transpose(2, 0, 1).reshape(KCH, P, 3 * Z) * s).astype(f8)
            wilc[l] = (Wilc[l, a].reshape(KCH, P, Z) * s).astype(f8)
        wlinT = Wlin[:, 0, :].T.reshape(KCH, P, T).astype(bf)
        in_maps.append(dict(xs=np.ascontiguousarray(xT[a * TSH:(a + 1) * TSH]),
                            w1=w1, wg=wg, wgp=wgp, wilc=wilc,
                            wsls=np.ascontiguousarray(wslT_full[a * LSH:(a + 1) * LSH]),
                            wlinT=wlinT))
    return in_maps


def kernel(x, Wx, Wh, b_lstm, Wg_h, Wg_p, bg, Wilc, bilc, Wsl, bsl, Wlin, blin,
           _nsteps=T):
    x = np.asarray(x, np.float32)
    for nm, b in (("b_lstm", b_lstm), ("bg", bg), ("bilc", bilc), ("bsl", bsl),
                  ("blin", blin)):
        assert not np.any(np.asarray(b)), f"nonzero bias {nm} unsupported"
    in_maps = _prep_inputs(x, np.asarray(Wx, np.float32), np.asarray(Wh, np.float32),
                           np.asarray(Wg_h, np.float32), np.asarray(Wg_p, np.float32),
                           np.asarray(Wilc, np.float32), np.asarray(Wsl, np.float32),
                           np.asarray(Wlin, np.float32))
    nc = build(_nsteps)
    res = run_bass_kernel_spmd(nc, in_maps, list(range(NC)))
    y = np.asarray(res.results[0]["y"], np.float32)  # (B, T)
    return np.ascontiguousarray(y.T[:, :, None])  # (T, B, 1)
